# revision 1
# baseline (speedup 1.0000x reference)
"""Bass/Trainium2 kernel for nn_BBBGraphConv (Bayesian GraphConv, DGL norm='both').

Computation (reference):
    W    = W_mu + W_eps * softplus(W_rho)
    bias = bias_mu + bias_eps * softplus(bias_rho)
    o    = clip(out_deg, 1)^-0.5 ; i = clip(in_deg, 1)^-0.5
    out  = segsum_dst((feat * o)[src]) @ W * i[:, None] + bias

Distribution: edges are bucketed by destination node; each of the 8 cores owns a
contiguous range of 12544 destination nodes (98 blocks x 128 dsts) and computes
its output rows exclusively (no all-reduce needed). The (pre-scaled) node
feature table is replicated to every core. Each core gathers the source rows of
its own edges with the gpsimd dma_gather instruction (the src id space is split
into 4 windows of 25088 rows so indices fit dma_gather's int16 format), reduces
them per destination with a TensorE one-hot-mask matmul (mask built from an
iota/is_equal compare on VectorE), projects through W, applies the dst-side
norm + bias, and writes its slice of the output.

Host-side work is limited to index-domain preprocessing (degree counts, sort,
bucketing/padding) and the out-degree pre-scaling of the feature table.
"""

import numpy as np
from contextlib import ExitStack

import concourse.bass as bass
import concourse.bacc as bacc
import concourse.tile as tile
from concourse import mybir
from concourse.bass_utils import run_bass_kernel_spmd

# Problem constants (hardcoded per the harness contract)
N_NODES = 100_000
N_EDGES = 1_600_000
C = 128          # in_ch == out_ch
P = 128          # partitions
N_CORES = 8
BLK = 128        # dst nodes per block
NB = 98          # blocks per core
D_CORE = NB * BLK          # 12544 dst rows per core
N_PAD = N_CORES * D_CORE   # 100352

NW = 4           # src windows (dma_gather indices are int16)
WROWS = N_PAD // NW        # 25088 rows per window


def _sb_layout(s_cap: int):
    """Blocks per (superblock, window) gather call. k*s_cap is capped at 35
    (num_idxs <= 4480 per dma_gather call, the hardware-validated size) and the
    tail is tapered so the post-gather compute tail is short."""
    k = max(1, min(7, 35 // s_cap))
    sizes = [k] * (NB // k)
    rem = NB - k * len(sizes)
    if rem:
        sizes.append(rem)
    tail = sizes.pop()
    while tail > 1:
        h = tail // 2
        sizes.append(tail - h)
        tail = h
    sizes.append(1)
    assert sum(sizes) == NB
    offs = [sum(sizes[:i]) for i in range(len(sizes))]
    return sizes, offs

TBL_DT = mybir.dt.float16
TBL_NP = np.float16

_CACHE: dict = {}


def _build_program(s_cap: int):
    """Build the SPMD Bass program (one graph, runs on all 8 cores).

    s_cap: groups (of 128 edge slots) per (block, window) section.
    """
    gpb = NW * s_cap            # groups per block
    slots_blk = gpb * BLK       # edge slots per block
    idx_f_blk = s_cap * BLK // 16        # idx free-dim per (block, window)
    idx_f_total = NB * NW * idx_f_blk
    SB_SIZES, SB_OFF = _sb_layout(s_cap)
    N_SB = len(SB_SIZES)
    f32 = mybir.dt.float32

    nc = bacc.Bacc("TRN2", target_bir_lowering=False, debug=False, num_swdge_queues=4)

    table = nc.dram_tensor("table", [N_PAD, C], TBL_DT, kind="ExternalInput").ap()
    idx_t = nc.dram_tensor("idx", [P, idx_f_total], mybir.dt.int16,
                           kind="ExternalInput").ap()
    rel_t = nc.dram_tensor("rel", [P, NB * gpb], TBL_DT, kind="ExternalInput").ap()
    iota_t = nc.dram_tensor("iota", [P, gpb * BLK], TBL_DT, kind="ExternalInput").ap()
    ivec_t = nc.dram_tensor("ivec", [P, NB], f32, kind="ExternalInput").ap()
    w_mu = nc.dram_tensor("w_mu", [C, C], f32, kind="ExternalInput").ap()
    w_rho = nc.dram_tensor("w_rho", [C, C], f32, kind="ExternalInput").ap()
    w_eps = nc.dram_tensor("w_eps", [C, C], f32, kind="ExternalInput").ap()
    b_mu = nc.dram_tensor("b_mu", [1, C], f32, kind="ExternalInput").ap()
    b_rho = nc.dram_tensor("b_rho", [1, C], f32, kind="ExternalInput").ap()
    b_eps = nc.dram_tensor("b_eps", [1, C], f32, kind="ExternalInput").ap()
    out = nc.dram_tensor("out", [D_CORE, C], f32, kind="ExternalOutput").ap()

    with tile.TileContext(nc) as tc, ExitStack() as ctx:
        const = ctx.enter_context(tc.tile_pool(name="const", bufs=1))
        gpool = ctx.enter_context(tc.tile_pool(name="gather", bufs=3))
        mpool = ctx.enter_context(tc.tile_pool(name="mask", bufs=3))
        apool = ctx.enter_context(tc.tile_pool(name="aggf", bufs=3))
        opool = ctx.enter_context(tc.tile_pool(name="ostage", bufs=3))
        pa_pool = ctx.enter_context(tc.tile_pool(name="pa", bufs=3, space="PSUM"))
        pb_pool = ctx.enter_context(tc.tile_pool(name="pb", bufs=2, space="PSUM"))
        pc_pool = ctx.enter_context(tc.tile_pool(name="pc", bufs=1, space="PSUM"))

        # --- resident inputs -------------------------------------------------
        # idx loaded per superblock (separate tiles) so the first gather
        # doesn't wait for the whole index upload
        idx_off = [0]
        for k in SB_SIZES:
            idx_off.append(idx_off[-1] + NW * k * idx_f_blk)
        idx_tiles = []
        for s in range(N_SB):
            t = const.tile([P, idx_off[s + 1] - idx_off[s]], mybir.dt.int16,
                           tag=f"idx{s}")
            nc.sync.dma_start(out=t[:], in_=idx_t[:, idx_off[s]:idx_off[s + 1]])
            idx_tiles.append(t)
        rel_sb = const.tile([P, NB * gpb], TBL_DT, tag="rel")
        nc.sync.dma_start(out=rel_sb[:], in_=rel_t[:])
        ivec_sb = const.tile([P, NB], f32, tag="ivec")
        nc.sync.dma_start(out=ivec_sb[:], in_=ivec_t[:])

        # --- W = W_mu + W_eps * softplus(W_rho) ------------------------------
        wmu_sb = const.tile([C, C], f32, tag="wmu")
        nc.sync.dma_start(out=wmu_sb[:], in_=w_mu[:])
        wrho_sb = const.tile([C, C], f32, tag="wrho")
        nc.sync.dma_start(out=wrho_sb[:], in_=w_rho[:])
        weps_sb = const.tile([C, C], f32, tag="weps")
        nc.sync.dma_start(out=weps_sb[:], in_=w_eps[:])
        w_sp = const.tile([C, C], f32, tag="wsp")
        nc.scalar.activation(w_sp[:], wrho_sb[:], mybir.ActivationFunctionType.Exp)
        nc.scalar.activation(w_sp[:], w_sp[:], mybir.ActivationFunctionType.Ln, bias=1.0)
        w_sb = const.tile([C, C], f32, tag="w")
        nc.vector.tensor_tensor(out=w_sb[:], in0=weps_sb[:], in1=w_sp[:], op=mybir.AluOpType.mult)
        nc.vector.tensor_tensor(out=w_sb[:], in0=w_sb[:], in1=wmu_sb[:], op=mybir.AluOpType.add)

        # --- bias tile [P, C]: every partition row holds the bias vector -----
        bmu_sb = const.tile([1, C], f32, tag="bmu")
        nc.sync.dma_start(out=bmu_sb[:], in_=b_mu[:])
        brho_sb = const.tile([1, C], f32, tag="brho")
        nc.sync.dma_start(out=brho_sb[:], in_=b_rho[:])
        beps_sb = const.tile([1, C], f32, tag="beps")
        nc.sync.dma_start(out=beps_sb[:], in_=b_eps[:])
        b_sp = const.tile([1, C], f32, tag="bsp")
        nc.scalar.activation(b_sp[:], brho_sb[:], mybir.ActivationFunctionType.Exp)
        nc.scalar.activation(b_sp[:], b_sp[:], mybir.ActivationFunctionType.Ln, bias=1.0)
        b_vec = const.tile([1, C], f32, tag="bvec")
        nc.vector.tensor_tensor(out=b_vec[:], in0=beps_sb[:], in1=b_sp[:], op=mybir.AluOpType.mult)
        nc.vector.tensor_tensor(out=b_vec[:], in0=b_vec[:], in1=bmu_sb[:], op=mybir.AluOpType.add)
        ones_1p = const.tile([1, C], f32, tag="ones")
        nc.vector.memset(ones_1p[:], 1.0)
        p_bias = pc_pool.tile([P, C], f32, tag="pbias")
        nc.tensor.matmul(out=p_bias[:], lhsT=ones_1p[:], rhs=b_vec[:], start=True, stop=True)
        bias_tile = const.tile([P, C], f32, tag="bias")
        nc.vector.tensor_copy(out=bias_tile[:], in_=p_bias[:])

        # --- iota over d within a block (host-provided constant) -------------
        iota_m = const.tile([P, slots_blk], TBL_DT, tag="iotam")
        nc.sync.dma_start(out=iota_m[:], in_=iota_t[:])
        iota3 = iota_m[:].rearrange("p (g d) -> p g d", g=gpb)

        # --- main loop over superblocks --------------------------------------
        for s in range(N_SB):
            k_sb = SB_SIZES[s]
            sb_groups = k_sb * s_cap
            g_tile = gpool.tile([P, sb_groups * NW * C], TBL_DT, tag="g")
            g3 = g_tile[:].rearrange("p (g c) -> p g c", c=C)
            for w in range(NW):
                call = w * k_sb * idx_f_blk
                nc.gpsimd.dma_gather(
                    out_ap=g3[:, w * sb_groups:(w + 1) * sb_groups, :],
                    in_ap=table[w * WROWS:(w + 1) * WROWS, :],
                    idxs_ap=idx_tiles[s][:, call:call + k_sb * idx_f_blk],
                    num_idxs=sb_groups * BLK,
                    num_idxs_reg=sb_groups * BLK,
                    elem_size=C,
                    queue_num=w,
                    single_packet=False,
                )
            ostage = opool.tile([P, k_sb * C], f32, tag="ostage")
            for bb in range(k_sb):
                b = SB_OFF[s] + bb
                mask = mpool.tile([P, slots_blk], TBL_DT, tag="mask")
                rel_b = rel_sb[:, b * gpb:(b + 1) * gpb].unsqueeze(2).to_broadcast(
                    [P, gpb, BLK]
                )
                nc.vector.tensor_tensor(
                    out=mask[:].rearrange("p (g d) -> p g d", g=gpb),
                    in0=iota3,
                    in1=rel_b,
                    op=mybir.AluOpType.is_equal,
                )
                pa = pa_pool.tile([C, BLK], f32, tag="pa")
                for j in range(gpb):
                    w, g = divmod(j, s_cap)
                    gsl = (w * k_sb + bb) * s_cap + g
                    nc.tensor.matmul(
                        out=pa[:],
                        lhsT=g_tile[:, gsl * C:(gsl + 1) * C],
                        rhs=mask[:, j * BLK:(j + 1) * BLK],
                        start=(j == 0),
                        stop=(j == gpb - 1),
                    )
                agg = apool.tile([C, BLK], f32, tag="agg")
                nc.scalar.activation(agg[:], pa[:], mybir.ActivationFunctionType.Copy)
                pb = pb_pool.tile([BLK, C], f32, tag="pb")
                nc.tensor.matmul(out=pb[:], lhsT=agg[:], rhs=w_sb[:], start=True, stop=True)
                nc.vector.scalar_tensor_tensor(
                    out=ostage[:, bb * C:(bb + 1) * C],
                    in0=pb[:],
                    scalar=ivec_sb[:, b:b + 1],
                    in1=bias_tile[:],
                    op0=mybir.AluOpType.mult,
                    op1=mybir.AluOpType.add,
                )
            dram_view = out[SB_OFF[s] * BLK:(SB_OFF[s] + k_sb) * BLK, :].rearrange(
                "(bb p) c -> p bb c", p=P
            )
            nc.sync.dma_start(
                out=dram_view, in_=ostage[:].rearrange("p (bb c) -> p bb c", bb=k_sb)
            )

    nc.compile()
    return nc


def _preprocess(feat, src, dst, W_mu, W_rho, bias_mu, bias_rho, W_eps, bias_eps):
    """Index-domain preprocessing + table pre-scaling. Returns per-core in_maps."""
    src = np.asarray(src).astype(np.int64)
    dst = np.asarray(dst).astype(np.int64)
    feat = np.asarray(feat, dtype=np.float32)

    out_deg = np.bincount(src, minlength=N_NODES).astype(np.float32)
    o = 1.0 / np.sqrt(np.maximum(out_deg, 1.0))
    in_deg = np.bincount(dst, minlength=N_NODES)
    ivec_full = (1.0 / np.sqrt(np.maximum(in_deg, 1.0))).astype(np.float32)

    table = np.zeros((N_PAD, C), TBL_NP)
    table[:N_NODES] = (feat * o[:, None]).astype(TBL_NP)

    blk = dst >> 7                      # global dst block, 0..783
    win = src // WROWS                  # src window, 0..3
    order = np.lexsort((src, win + np.int64(NW) * blk))
    sblk = blk[order]
    swin = win[order]
    ss = src[order]
    sd = dst[order]

    n_blocks = N_CORES * NB
    sec = sblk * NW + swin              # global (block, window) section id
    sec_cnt = np.bincount(sec, minlength=n_blocks * NW)
    s_cap = int(np.ceil(sec_cnt.max() / BLK))
    sec_slots = s_cap * BLK

    starts = np.zeros(n_blocks * NW + 1, np.int64)
    np.cumsum(sec_cnt, out=starts[1:])
    pos = np.arange(len(ss), dtype=np.int64) - starts[sec]

    # slot id in (core, superblock, window, block-in-sb, group, lane) order
    SB_SIZES, SB_OFF = _sb_layout(s_cap)
    s_of_b = np.zeros(NB, np.int64)
    b_in_of_b = np.zeros(NB, np.int64)
    for s, k in enumerate(SB_SIZES):
        s_of_b[SB_OFF[s]:SB_OFF[s] + k] = s
        b_in_of_b[SB_OFF[s]:SB_OFF[s] + k] = np.arange(k)
    sb_off_arr = np.asarray(SB_OFF, np.int64)
    k_arr = np.asarray(SB_SIZES, np.int64)
    core = sblk // NB
    b_loc = sblk % NB
    s_id = s_of_b[b_loc]
    sec_idx = NW * sb_off_arr[s_id] + swin * k_arr[s_id] + b_in_of_b[b_loc]
    slots_core = NB * NW * sec_slots
    slot = core * slots_core + sec_idx * sec_slots + pos

    idx_all = np.zeros(N_CORES * slots_core, np.int16)    # pad -> row 0 of window
    idx_all[slot] = (ss - swin * WROWS).astype(np.int16)

    # rel in (core, block, window, group, lane) order
    rel_slot = (sblk * NW + swin) * sec_slots + pos
    rel_all = np.full(n_blocks * NW * sec_slots, -1.0, TBL_NP)
    rel_all[rel_slot] = (sd & 127).astype(TBL_NP)

    ivec_pad = np.ones(N_PAD, np.float32)
    ivec_pad[:N_NODES] = ivec_full

    gpb = NW * s_cap
    iota_np = np.tile(np.arange(BLK, dtype=TBL_NP), gpb)[None, :].repeat(P, 0)
    iota_np = np.ascontiguousarray(iota_np)

    cc = np.ascontiguousarray
    in_maps = []
    for c in range(N_CORES):
        # idx: 16-wrapped int16 (valid per call since call sizes are multiples
        # of 16), replicated across the 8 Q7 pairs
        idx_c = idx_all[c * slots_core:(c + 1) * slots_core]
        idx_tile = np.tile(idx_c.reshape(-1, 16).T, (8, 1))
        rel_c = rel_all[c * NB * NW * sec_slots:(c + 1) * NB * NW * sec_slots]
        in_maps.append({
            "table": table,
            "iota": iota_np,
            "idx": cc(idx_tile),
            "rel": cc(rel_c.reshape(-1, P).T),
            "ivec": cc(ivec_pad[c * D_CORE:(c + 1) * D_CORE].reshape(NB, P).T),
            "w_mu": np.asarray(W_mu, np.float32),
            "w_rho": np.asarray(W_rho, np.float32),
            "w_eps": np.asarray(W_eps, np.float32),
            "b_mu": np.asarray(bias_mu, np.float32).reshape(1, C),
            "b_rho": np.asarray(bias_rho, np.float32).reshape(1, C),
            "b_eps": np.asarray(bias_eps, np.float32).reshape(1, C),
        })
    return in_maps, s_cap


def kernel(**inputs) -> np.ndarray:
    in_maps, s_cap = _preprocess(**inputs)
    if s_cap not in _CACHE:
        _CACHE[s_cap] = _build_program(s_cap)
    nc = _CACHE[s_cap]
    res = run_bass_kernel_spmd(nc, in_maps, core_ids=list(range(N_CORES)))
    parts = [res.results[c]["out"] for c in range(N_CORES)]
    return np.concatenate(parts, axis=0)[:N_NODES]



# revision 2
# speedup vs baseline: 1.0505x; 1.0505x over previous
"""Bass/Trainium2 kernel for nn_BBBGraphConv (Bayesian GraphConv, DGL norm='both').

Computation (reference):
    W    = W_mu + W_eps * softplus(W_rho)
    bias = bias_mu + bias_eps * softplus(bias_rho)
    o    = clip(out_deg, 1)^-0.5 ; i = clip(in_deg, 1)^-0.5
    out  = segsum_dst((feat * o)[src]) @ W * i[:, None] + bias

Distribution: edges are bucketed by destination node; each of the 8 cores owns
98 blocks of 128 dst nodes (12544 rows) and computes its output rows
exclusively. The pre-scaled node feature table (fp16) is replicated; each core
gathers the source rows of its own edges with gpsimd dma_gather (src ids split
into 4 windows of 25088 rows for the int16 index format), reduces them per
destination with TensorE one-hot-mask matmuls, projects through W (fp16),
applies the dst-side norm + bias, and writes its slice of the output.

v1 improvements over the fixed-s_cap baseline:
  * variable groups per (block, window) section -- ceil(cnt/128) instead of a
    global max -- cuts gather padding from ~25% to ~14%. The SPMD program is
    kept core-independent by sorting each core's blocks by their group-count
    signature and building the program for the element-wise max; outputs are
    un-permuted on the host.
  * the one-hot mask is built with is_equal on operands whose innermost dims
    are packed (rel duplicated in pairs, viewed [p,g,64,2]) to hit the DVE
    2x_1p fast path.
  * the post-aggregation projection runs in fp16 (1 cycle/row on the PE)
    instead of fp32 (4 cycles/row).
"""

import numpy as np
from contextlib import ExitStack

import concourse.bass as bass
import concourse.bacc as bacc
import concourse.tile as tile
from concourse import mybir
from concourse.bass_utils import run_bass_kernel_spmd

# Problem constants (hardcoded per the harness contract)
N_NODES = 100_000
N_EDGES = 1_600_000
C = 128          # in_ch == out_ch
P = 128          # partitions
N_CORES = 8
BLK = 128        # dst nodes per block
NB = 98          # blocks per core
D_CORE = NB * BLK          # 12544 dst rows per core
N_PAD = N_CORES * D_CORE   # 100352

NW = 4           # src windows (dma_gather indices are int16)
WROWS = N_PAD // NW        # 25088 rows per window

CALL_CAP = 35    # max groups per dma_gather call (num_idxs <= 4480)
GSB_CAP = 132    # max groups per superblock (g_tile sizing)

TBL_DT = mybir.dt.float16
TBL_NP = np.float16

_CACHE: dict = {}


def _plan_superblocks(g_uni):
    """Pack blocks into superblocks subject to per-window call caps and the
    total-group cap. Returns list of (k0, k1) block ranges."""
    sbs = []
    k0 = 0
    wsum = np.zeros(NW, np.int64)
    tsum = 0
    for k in range(NB):
        gw = g_uni[k]
        if k > k0 and (np.any(wsum + gw > CALL_CAP) or tsum + gw.sum() > GSB_CAP):
            sbs.append((k0, k))
            k0 = k
            wsum = np.zeros(NW, np.int64)
            tsum = 0
        wsum += gw
        tsum += int(gw.sum())
    sbs.append((k0, NB))
    return sbs


def _build_program(key):
    """Build the SPMD Bass program (one graph, runs on all 8 cores).

    key: (g_uni_flat, sbs) -- the shared structural layout.
    """
    g_uni_flat, sbs = key
    g_uni = np.asarray(g_uni_flat, np.int64).reshape(NB, NW)
    gbtot = g_uni.sum(axis=1)            # mask groups per block
    GB_MAX = int(gbtot.max())
    G_TOTAL = int(gbtot.sum())           # total groups per core
    idx_f_total = G_TOTAL * BLK // 16    # int16 idx values per partition row
    f32 = mybir.dt.float32

    # per-superblock group layout
    sb_meta = []
    colbase = 0
    idxbase = 0
    relbase = 0
    for (k0, k1) in sbs:
        gw = g_uni[k0:k1].sum(axis=0)    # groups per window in this sb
        gsb = int(gw.sum())
        woff = np.concatenate([[0], np.cumsum(gw)])[:NW]
        # column of group (k, w, j) within the sb tile:
        #   woff[w] + sum_{k' in [k0,k)} g_uni[k', w] + j
        sb_meta.append(dict(k0=k0, k1=k1, gw=gw, gsb=gsb, woff=woff,
                            colbase=colbase, idxbase=idxbase, relbase=relbase))
        colbase += gsb
        idxbase += gsb * BLK // 16
        relbase += int(gbtot[k0:k1].sum())
    assert colbase == G_TOTAL

    nc = bacc.Bacc("TRN2", target_bir_lowering=False, debug=False, num_swdge_queues=4)

    table = nc.dram_tensor("table", [N_PAD, C], TBL_DT, kind="ExternalInput").ap()
    idx_t = nc.dram_tensor("idx", [P, idx_f_total], mybir.dt.int16,
                           kind="ExternalInput").ap()
    rel_t = nc.dram_tensor("rel", [P, G_TOTAL * 2], TBL_DT, kind="ExternalInput").ap()
    iota_t = nc.dram_tensor("iota", [P, GB_MAX * BLK], TBL_DT, kind="ExternalInput").ap()
    ivec_t = nc.dram_tensor("ivec", [P, NB], f32, kind="ExternalInput").ap()
    w_mu = nc.dram_tensor("w_mu", [C, C], f32, kind="ExternalInput").ap()
    w_rho = nc.dram_tensor("w_rho", [C, C], f32, kind="ExternalInput").ap()
    w_eps = nc.dram_tensor("w_eps", [C, C], f32, kind="ExternalInput").ap()
    b_mu = nc.dram_tensor("b_mu", [1, C], f32, kind="ExternalInput").ap()
    b_rho = nc.dram_tensor("b_rho", [1, C], f32, kind="ExternalInput").ap()
    b_eps = nc.dram_tensor("b_eps", [1, C], f32, kind="ExternalInput").ap()
    out = nc.dram_tensor("out", [D_CORE, C], f32, kind="ExternalOutput").ap()

    GSB_MAX = max(m["gsb"] for m in sb_meta)
    KB_MAX = max(m["k1"] - m["k0"] for m in sb_meta)

    with tile.TileContext(nc) as tc, ExitStack() as ctx:
        const = ctx.enter_context(tc.tile_pool(name="const", bufs=1))
        gpool = ctx.enter_context(tc.tile_pool(name="gather", bufs=3))
        mpool = ctx.enter_context(tc.tile_pool(name="mask", bufs=4))
        apool = ctx.enter_context(tc.tile_pool(name="aggf", bufs=3))
        opool = ctx.enter_context(tc.tile_pool(name="ostage", bufs=3))
        pa_pool = ctx.enter_context(tc.tile_pool(name="pa", bufs=3, space="PSUM"))
        pb_pool = ctx.enter_context(tc.tile_pool(name="pb", bufs=2, space="PSUM"))
        pc_pool = ctx.enter_context(tc.tile_pool(name="pc", bufs=1, space="PSUM"))

        # --- resident inputs -------------------------------------------------
        # idx loaded per superblock (separate tiles) so the first gather
        # doesn't wait for the whole index upload
        idx_tiles = []
        for s, m in enumerate(sb_meta):
            n = m["gsb"] * BLK // 16
            t = const.tile([P, n], mybir.dt.int16, tag=f"idx{s}")
            nc.sync.dma_start(out=t[:], in_=idx_t[:, m["idxbase"]:m["idxbase"] + n])
            idx_tiles.append(t)
        rel_sb = const.tile([P, G_TOTAL * 2], TBL_DT, tag="rel")
        nc.sync.dma_start(out=rel_sb[:], in_=rel_t[:])
        ivec_sb = const.tile([P, NB], f32, tag="ivec")
        nc.sync.dma_start(out=ivec_sb[:], in_=ivec_t[:])
        iota_m = const.tile([P, GB_MAX * BLK], TBL_DT, tag="iotam")
        nc.sync.dma_start(out=iota_m[:], in_=iota_t[:])

        # --- W = W_mu + W_eps * softplus(W_rho), cast to fp16 ---------------
        wmu_sb = const.tile([C, C], f32, tag="wmu")
        nc.sync.dma_start(out=wmu_sb[:], in_=w_mu[:])
        wrho_sb = const.tile([C, C], f32, tag="wrho")
        nc.sync.dma_start(out=wrho_sb[:], in_=w_rho[:])
        weps_sb = const.tile([C, C], f32, tag="weps")
        nc.sync.dma_start(out=weps_sb[:], in_=w_eps[:])
        w_sp = const.tile([C, C], f32, tag="wsp")
        nc.scalar.activation(w_sp[:], wrho_sb[:], mybir.ActivationFunctionType.Exp)
        nc.scalar.activation(w_sp[:], w_sp[:], mybir.ActivationFunctionType.Ln, bias=1.0)
        w_f32 = const.tile([C, C], f32, tag="wf32")
        nc.vector.tensor_tensor(out=w_f32[:], in0=weps_sb[:], in1=w_sp[:], op=mybir.AluOpType.mult)
        nc.vector.tensor_tensor(out=w_f32[:], in0=w_f32[:], in1=wmu_sb[:], op=mybir.AluOpType.add)
        w16 = const.tile([C, C], TBL_DT, tag="w16")
        nc.vector.tensor_copy(out=w16[:], in_=w_f32[:])

        # --- bias tile [P, C]: every partition row holds the bias vector -----
        bmu_sb = const.tile([1, C], f32, tag="bmu")
        nc.sync.dma_start(out=bmu_sb[:], in_=b_mu[:])
        brho_sb = const.tile([1, C], f32, tag="brho")
        nc.sync.dma_start(out=brho_sb[:], in_=b_rho[:])
        beps_sb = const.tile([1, C], f32, tag="beps")
        nc.sync.dma_start(out=beps_sb[:], in_=b_eps[:])
        b_sp = const.tile([1, C], f32, tag="bsp")
        nc.scalar.activation(b_sp[:], brho_sb[:], mybir.ActivationFunctionType.Exp)
        nc.scalar.activation(b_sp[:], b_sp[:], mybir.ActivationFunctionType.Ln, bias=1.0)
        b_vec = const.tile([1, C], f32, tag="bvec")
        nc.vector.tensor_tensor(out=b_vec[:], in0=beps_sb[:], in1=b_sp[:], op=mybir.AluOpType.mult)
        nc.vector.tensor_tensor(out=b_vec[:], in0=b_vec[:], in1=bmu_sb[:], op=mybir.AluOpType.add)
        ones_1p = const.tile([1, C], f32, tag="ones")
        nc.vector.memset(ones_1p[:], 1.0)
        p_bias = pc_pool.tile([P, C], f32, tag="pbias")
        nc.tensor.matmul(out=p_bias[:], lhsT=ones_1p[:], rhs=b_vec[:], start=True, stop=True)
        bias_tile = const.tile([P, C], f32, tag="bias")
        nc.vector.tensor_copy(out=bias_tile[:], in_=p_bias[:])

        # --- main loop over superblocks --------------------------------------
        for s, m in enumerate(sb_meta):
            k0, k1, gw, gsb, woff = m["k0"], m["k1"], m["gw"], m["gsb"], m["woff"]
            kb = k1 - k0
            g_tile = gpool.tile([P, GSB_MAX * C], TBL_DT, tag="g")
            g3 = g_tile[:].rearrange("p (g c) -> p g c", c=C)
            icall = 0
            for w in range(NW):
                ngw = int(gw[w])
                if ngw == 0:
                    continue
                nc.gpsimd.dma_gather(
                    out_ap=g3[:, woff[w]:woff[w] + ngw, :],
                    in_ap=table[w * WROWS:(w + 1) * WROWS, :],
                    idxs_ap=idx_tiles[s][:, icall:icall + ngw * BLK // 16],
                    num_idxs=ngw * BLK,
                    num_idxs_reg=ngw * BLK,
                    elem_size=C,
                    queue_num=w,
                    single_packet=False,
                )
                icall += ngw * BLK // 16
            ostage = opool.tile([P, KB_MAX * C], f32, tag="ostage")
            boff = np.zeros(NW, np.int64)
            relcol = m["relbase"]
            for kk in range(kb):
                k = k0 + kk
                gb = int(gbtot[k])
                mask = mpool.tile([P, GB_MAX * BLK], TBL_DT, tag="mask")
                # one-hot mask via is_equal; rel is stored as duplicated
                # pairs so every operand's innermost dim is packed (DVE 2x).
                nc.vector.tensor_tensor(
                    out=mask[:, :gb * BLK].rearrange(
                        "p (g x y) -> p g x y", g=gb, y=2),
                    in0=iota_m[:, :gb * BLK].rearrange(
                        "p (g x y) -> p g x y", g=gb, y=2),
                    in1=rel_sb[:, relcol * 2:(relcol + gb) * 2].rearrange(
                        "p (g y) -> p g y", y=2).unsqueeze(2).to_broadcast(
                        [P, gb, BLK // 2, 2]),
                    op=mybir.AluOpType.is_equal,
                )
                pa = pa_pool.tile([C, BLK], f32, tag="pa")
                j = 0
                for w in range(NW):
                    for jj in range(int(g_uni[k, w])):
                        col = int(woff[w] + boff[w] + jj)
                        nc.tensor.matmul(
                            out=pa[:],
                            lhsT=g_tile[:, col * C:(col + 1) * C],
                            rhs=mask[:, j * BLK:(j + 1) * BLK],
                            start=(j == 0),
                            stop=(j == gb - 1),
                        )
                        j += 1
                agg = apool.tile([C, BLK], TBL_DT, tag="agg")
                nc.scalar.activation(agg[:], pa[:], mybir.ActivationFunctionType.Copy)
                pb = pb_pool.tile([BLK, C], f32, tag="pb")
                nc.tensor.matmul(out=pb[:], lhsT=agg[:], rhs=w16[:], start=True, stop=True)
                nc.vector.scalar_tensor_tensor(
                    out=ostage[:, kk * C:(kk + 1) * C],
                    in0=pb[:],
                    scalar=ivec_sb[:, k:k + 1],
                    in1=bias_tile[:],
                    op0=mybir.AluOpType.mult,
                    op1=mybir.AluOpType.add,
                )
                relcol += gb
                boff += g_uni[k]
            dram_view = out[k0 * BLK:k1 * BLK, :].rearrange(
                "(bb p) c -> p bb c", p=P
            )
            nc.sync.dma_start(
                out=dram_view,
                in_=ostage[:, :kb * C].rearrange("p (bb c) -> p bb c", bb=kb),
            )

    nc.compile()
    return nc


def _preprocess(feat, src, dst, W_mu, W_rho, bias_mu, bias_rho, W_eps, bias_eps):
    """Index-domain preprocessing + table pre-scaling. Returns (in_maps, key)."""
    src = np.asarray(src).astype(np.int64)
    dst = np.asarray(dst).astype(np.int64)
    feat = np.asarray(feat, dtype=np.float32)

    out_deg = np.bincount(src, minlength=N_NODES).astype(np.float32)
    o = 1.0 / np.sqrt(np.maximum(out_deg, 1.0))
    in_deg = np.bincount(dst, minlength=N_NODES)
    ivec_full = (1.0 / np.sqrt(np.maximum(in_deg, 1.0))).astype(np.float32)

    table = np.zeros((N_PAD, C), TBL_NP)
    table[:N_NODES] = (feat * o[:, None]).astype(TBL_NP)

    blk = dst >> 7                       # global dst block, 0..783
    win = src // WROWS                   # src window, 0..3
    core = blk // NB
    b_loc = blk % NB

    # --- per-(core, block, window) group counts and core matching ------------
    sec_cnt = np.bincount(blk * NW + win, minlength=NB * N_CORES * NW)
    g_all = -(-sec_cnt // BLK).reshape(N_CORES, NB, NW)     # ceil(cnt/128)
    gb_all = g_all.sum(axis=2)                              # [cores, NB]
    # sort each core's blocks by signature (desc) so the shared structural
    # layout (element-wise max across cores) wastes little
    perm = np.zeros((N_CORES, NB), np.int64)   # structural k -> local block
    rank = np.zeros((N_CORES, NB), np.int64)   # local block -> structural k
    for c in range(N_CORES):
        key_arr = np.lexsort(tuple(-g_all[c, :, w] for w in range(NW - 1, -1, -1))
                             + (-gb_all[c],))
        perm[c] = key_arr
        rank[c, key_arr] = np.arange(NB)
    g_sorted = np.take_along_axis(g_all, perm[:, :, None], axis=1)
    g_uni = g_sorted.max(axis=0)                            # [NB, NW]
    gbtot = g_uni.sum(axis=1)
    G_TOTAL = int(gbtot.sum())
    GB_MAX = int(gbtot.max())

    sbs = _plan_superblocks(g_uni)
    key = (tuple(g_uni.ravel().tolist()), tuple(sbs))

    # --- structural slot bases ----------------------------------------------
    # column base of section (k, w): within sb: woff[w] + sum_{k' in sb, k'<k} g_uni[k', w]
    col_base = np.zeros((NB, NW), np.int64)
    colbase = 0
    for (k0, k1) in sbs:
        gw = g_uni[k0:k1].sum(axis=0)
        woff = np.concatenate([[0], np.cumsum(gw)])[:NW]
        run = np.zeros(NW, np.int64)
        for k in range(k0, k1):
            col_base[k] = colbase + woff + run
            run += g_uni[k]
        colbase += int(gw.sum())
    assert colbase == G_TOTAL
    # rel group base of block k (block-major): cumsum of gbtot; within a block
    # groups are ordered (w, j)
    rel_base = np.concatenate([[0], np.cumsum(gbtot)])[:NB]
    wg_off = np.concatenate([np.zeros((NB, 1), np.int64),
                             np.cumsum(g_uni, axis=1)[:, :NW - 1]], axis=1)

    # --- per-edge slot computation ------------------------------------------
    k_rank = rank[core, b_loc]                     # structural block of edge
    seckey = (core * NB + k_rank) * NW + win
    order = np.argsort(seckey, kind="stable")
    sk = seckey[order]
    ss = src[order]
    sd = dst[order]
    cnt = np.bincount(seckey, minlength=N_CORES * NB * NW)
    starts = np.zeros(N_CORES * NB * NW + 1, np.int64)
    np.cumsum(cnt, out=starts[1:])
    pos = np.arange(len(ss), dtype=np.int64) - starts[sk]

    kr = (sk // NW) % NB
    wr = sk % NW
    cr = sk // (NW * NB)
    slots_core = G_TOTAL * BLK
    slot = cr * slots_core + (col_base[kr, wr] * BLK) + pos
    idx_all = np.zeros(N_CORES * slots_core, np.int16)     # pad -> row 0
    idx_all[slot] = (ss - wr * WROWS).astype(np.int16)

    rel_slot = cr * slots_core + (rel_base[kr] + wg_off[kr, wr]) * BLK + pos
    rel_all = np.full(N_CORES * slots_core, -1.0, TBL_NP)
    rel_all[rel_slot] = (sd & 127).astype(TBL_NP)

    ivec_pad = np.ones(N_PAD, np.float32)
    ivec_pad[:N_NODES] = ivec_full

    iota_np = np.tile(np.arange(BLK, dtype=TBL_NP), GB_MAX)[None, :].repeat(P, 0)
    iota_np = np.ascontiguousarray(iota_np)

    cc = np.ascontiguousarray
    in_maps = []
    for c in range(N_CORES):
        # idx: 16-wrapped int16 (valid per call since call sizes are multiples
        # of 16), replicated across the 8 Q7 pairs
        idx_c = idx_all[c * slots_core:(c + 1) * slots_core]
        idx_tile = np.tile(idx_c.reshape(-1, 16).T, (8, 1))
        # rel: [P, G, 2] with each group's lane id duplicated
        rel_c = rel_all[c * slots_core:(c + 1) * slots_core].reshape(G_TOTAL, P).T
        rel2 = np.repeat(rel_c[:, :, None], 2, axis=2).reshape(P, G_TOTAL * 2)
        # ivec permuted to structural block order
        iv = ivec_pad[c * D_CORE:(c + 1) * D_CORE].reshape(NB, P)[perm[c]].T
        in_maps.append({
            "table": table,
            "iota": iota_np,
            "idx": cc(idx_tile),
            "rel": cc(rel2),
            "ivec": cc(iv),
            "w_mu": np.asarray(W_mu, np.float32),
            "w_rho": np.asarray(W_rho, np.float32),
            "w_eps": np.asarray(W_eps, np.float32),
            "b_mu": np.asarray(bias_mu, np.float32).reshape(1, C),
            "b_rho": np.asarray(bias_rho, np.float32).reshape(1, C),
            "b_eps": np.asarray(bias_eps, np.float32).reshape(1, C),
        })
    # stash the permutation for output unshuffling
    in_maps[0]["_perm"] = perm
    return in_maps, key


def kernel(**inputs) -> np.ndarray:
    in_maps, key = _preprocess(**inputs)
    perm = in_maps[0].pop("_perm")
    if key not in _CACHE:
        _CACHE[key] = _build_program(key)
    nc = _CACHE[key]
    res = run_bass_kernel_spmd(nc, in_maps, core_ids=list(range(N_CORES)))
    lanes = np.arange(P, dtype=np.int64)
    out_full = np.empty((N_PAD, C), np.float32)
    for c in range(N_CORES):
        rows = (perm[c][:, None] * P + lanes[None, :]).ravel()
        out_full[c * D_CORE + rows] = res.results[c]["out"]
    return out_full[:N_NODES]


# revision 3
# speedup vs baseline: 1.2875x; 1.2256x over previous
"""Bass/Trainium2 kernel for nn_BBBGraphConv (Bayesian GraphConv, DGL norm='both').

Computation (reference):
    W    = W_mu + W_eps * softplus(W_rho)
    bias = bias_mu + bias_eps * softplus(bias_rho)
    o    = clip(out_deg, 1)^-0.5 ; i = clip(in_deg, 1)^-0.5
    out  = segsum_dst((feat * o)[src]) @ W * i[:, None] + bias

Distribution: dst nodes are assigned to (core, block, lane) by a host-side
load balancer so that every (block, src-window) section holds at most
512 edges (640 for one designated overflow block per core).  This gives a
single uniform SPMD program: 97 blocks x 4 groups per window plus one block
x 5 groups per window per core, with ~0.6% gather padding.  Each of the 8
cores owns 98 blocks of 128 dst nodes and computes their output rows
exclusively (no collective needed).  The pre-scaled fp16 feature table is
replicated; each core gathers the source rows of its edges with gpsimd
dma_gather (the src id space is split into 4 windows of 25088 rows for the
int16 index format), reduces them per destination with TensorE one-hot-mask
matmuls, projects through W in fp16, applies dst-side norm + bias, and
writes its slice.  The host un-permutes the rows afterwards.

The gather descriptor supply (Q7 SWDGE descriptor generation, ~7.3ns per
index per queue across 4 queues) is the end-to-end bottleneck, so the
design minimizes the gathered index count above all else.

The one-hot masks are built with is_equal on operands whose innermost dims
are packed (rel duplicated in pairs, viewed [p,g,64,2]) to hit the DVE
2x_1p fast path; the projection runs in fp16 (1 cycle/row on the PE).
"""

import numpy as np
from contextlib import ExitStack

import concourse.bass as bass
import concourse.bacc as bacc
import concourse.tile as tile
from concourse import mybir
from concourse.bass_utils import run_bass_kernel_spmd

# Problem constants (hardcoded per the harness contract)
N_NODES = 100_000
N_EDGES = 1_600_000
C = 128          # in_ch == out_ch
P = 128          # partitions
N_CORES = 8
BLK = 128        # dst nodes per block
NB = 98          # blocks per core
D_CORE = NB * BLK          # 12544 dst rows per core
N_PAD = N_CORES * D_CORE   # 100352
NBLK = N_CORES * NB        # 784 blocks total

NW = 4           # src windows (dma_gather indices are int16)
WROWS = N_PAD // NW        # 25088 rows per window

CALL_CAP = 35    # max groups per dma_gather call (num_idxs <= 4480)

TBL_DT = mybir.dt.float16
TBL_NP = np.float16

_CACHE: dict = {}


def _plan_superblocks(g_uni):
    """Pack blocks into superblocks subject to the per-window call cap, with a
    tapered tail so the final drain+compute is short."""
    sbs = []
    k0 = 0
    wsum = np.zeros(NW, np.int64)
    for k in range(NB):
        gw = g_uni[k]
        # taper: last blocks in smaller superblocks
        cap = CALL_CAP if k < NB - 4 else 12
        if k > k0 and np.any(wsum + gw > cap):
            sbs.append((k0, k))
            k0 = k
            wsum = np.zeros(NW, np.int64)
        wsum += gw
    sbs.append((k0, NB))
    return sbs


def _build_program(key):
    """Build the SPMD Bass program (one graph, runs on all 8 cores)."""
    g_uni_flat, sbs = key
    g_uni = np.asarray(g_uni_flat, np.int64).reshape(NB, NW)
    gbtot = g_uni.sum(axis=1)            # mask groups per block
    GB_MAX = int(gbtot.max())
    G_TOTAL = int(gbtot.sum())           # total groups per core
    idx_f_total = G_TOTAL * BLK // 16    # int16 idx values per partition row
    f32 = mybir.dt.float32

    sb_meta = []
    colbase = 0
    idxbase = 0
    relbase = 0
    for (k0, k1) in sbs:
        gw = g_uni[k0:k1].sum(axis=0)
        gsb = int(gw.sum())
        woff = np.concatenate([[0], np.cumsum(gw)])[:NW]
        sb_meta.append(dict(k0=k0, k1=k1, gw=gw, gsb=gsb, woff=woff,
                            colbase=colbase, idxbase=idxbase, relbase=relbase))
        colbase += gsb
        idxbase += gsb * BLK // 16
        relbase += int(gbtot[k0:k1].sum())
    assert colbase == G_TOTAL

    nc = bacc.Bacc("TRN2", target_bir_lowering=False, debug=False, num_swdge_queues=4)

    table = nc.dram_tensor("table", [N_PAD, C], TBL_DT, kind="ExternalInput").ap()
    idx_t = nc.dram_tensor("idx", [P, idx_f_total], mybir.dt.int16,
                           kind="ExternalInput").ap()
    rel_t = nc.dram_tensor("rel", [P, G_TOTAL * 2], TBL_DT, kind="ExternalInput").ap()
    iota_t = nc.dram_tensor("iota", [P, GB_MAX * BLK], TBL_DT, kind="ExternalInput").ap()
    ivec_t = nc.dram_tensor("ivec", [P, NB], f32, kind="ExternalInput").ap()
    w_mu = nc.dram_tensor("w_mu", [C, C], f32, kind="ExternalInput").ap()
    w_rho = nc.dram_tensor("w_rho", [C, C], f32, kind="ExternalInput").ap()
    w_eps = nc.dram_tensor("w_eps", [C, C], f32, kind="ExternalInput").ap()
    b_mu = nc.dram_tensor("b_mu", [1, C], f32, kind="ExternalInput").ap()
    b_rho = nc.dram_tensor("b_rho", [1, C], f32, kind="ExternalInput").ap()
    b_eps = nc.dram_tensor("b_eps", [1, C], f32, kind="ExternalInput").ap()
    out = nc.dram_tensor("out", [D_CORE, C], f32, kind="ExternalOutput").ap()

    GSB_MAX = max(m["gsb"] for m in sb_meta)
    KB_MAX = max(m["k1"] - m["k0"] for m in sb_meta)

    with tile.TileContext(nc) as tc, ExitStack() as ctx:
        const = ctx.enter_context(tc.tile_pool(name="const", bufs=1))
        gpool = ctx.enter_context(tc.tile_pool(name="gather", bufs=3))
        mpool = ctx.enter_context(tc.tile_pool(name="mask", bufs=4))
        apool = ctx.enter_context(tc.tile_pool(name="aggf", bufs=3))
        opool = ctx.enter_context(tc.tile_pool(name="ostage", bufs=3))
        pa_pool = ctx.enter_context(tc.tile_pool(name="pa", bufs=3, space="PSUM"))
        pb_pool = ctx.enter_context(tc.tile_pool(name="pb", bufs=2, space="PSUM"))
        pc_pool = ctx.enter_context(tc.tile_pool(name="pc", bufs=1, space="PSUM"))

        # --- resident inputs -------------------------------------------------
        idx_tiles = []
        for s, m in enumerate(sb_meta):
            n = m["gsb"] * BLK // 16
            t = const.tile([P, n], mybir.dt.int16, tag=f"idx{s}")
            nc.sync.dma_start(out=t[:], in_=idx_t[:, m["idxbase"]:m["idxbase"] + n])
            idx_tiles.append(t)
        rel_sb = const.tile([P, G_TOTAL * 2], TBL_DT, tag="rel")
        nc.sync.dma_start(out=rel_sb[:], in_=rel_t[:])
        ivec_sb = const.tile([P, NB], f32, tag="ivec")
        nc.sync.dma_start(out=ivec_sb[:], in_=ivec_t[:])
        iota_m = const.tile([P, GB_MAX * BLK], TBL_DT, tag="iotam")
        nc.sync.dma_start(out=iota_m[:], in_=iota_t[:])

        # --- W = W_mu + W_eps * softplus(W_rho), cast to fp16 ---------------
        wmu_sb = const.tile([C, C], f32, tag="wmu")
        nc.sync.dma_start(out=wmu_sb[:], in_=w_mu[:])
        wrho_sb = const.tile([C, C], f32, tag="wrho")
        nc.sync.dma_start(out=wrho_sb[:], in_=w_rho[:])
        weps_sb = const.tile([C, C], f32, tag="weps")
        nc.sync.dma_start(out=weps_sb[:], in_=w_eps[:])
        w_sp = const.tile([C, C], f32, tag="wsp")
        nc.scalar.activation(w_sp[:], wrho_sb[:], mybir.ActivationFunctionType.Exp)
        nc.scalar.activation(w_sp[:], w_sp[:], mybir.ActivationFunctionType.Ln, bias=1.0)
        w_f32 = const.tile([C, C], f32, tag="wf32")
        nc.vector.tensor_tensor(out=w_f32[:], in0=weps_sb[:], in1=w_sp[:], op=mybir.AluOpType.mult)
        nc.vector.tensor_tensor(out=w_f32[:], in0=w_f32[:], in1=wmu_sb[:], op=mybir.AluOpType.add)
        w16 = const.tile([C, C], TBL_DT, tag="w16")
        nc.scalar.activation(w16[:], w_f32[:], mybir.ActivationFunctionType.Copy)

        # --- bias tile [P, C]: every partition row holds the bias vector -----
        bmu_sb = const.tile([1, C], f32, tag="bmu")
        nc.sync.dma_start(out=bmu_sb[:], in_=b_mu[:])
        brho_sb = const.tile([1, C], f32, tag="brho")
        nc.sync.dma_start(out=brho_sb[:], in_=b_rho[:])
        beps_sb = const.tile([1, C], f32, tag="beps")
        nc.sync.dma_start(out=beps_sb[:], in_=b_eps[:])
        b_sp = const.tile([1, C], f32, tag="bsp")
        nc.scalar.activation(b_sp[:], brho_sb[:], mybir.ActivationFunctionType.Exp)
        nc.scalar.activation(b_sp[:], b_sp[:], mybir.ActivationFunctionType.Ln, bias=1.0)
        b_vec = const.tile([1, C], f32, tag="bvec")
        nc.vector.tensor_tensor(out=b_vec[:], in0=beps_sb[:], in1=b_sp[:], op=mybir.AluOpType.mult)
        nc.vector.tensor_tensor(out=b_vec[:], in0=b_vec[:], in1=bmu_sb[:], op=mybir.AluOpType.add)
        ones_1p = const.tile([1, C], f32, tag="ones")
        nc.vector.memset(ones_1p[:], 1.0)
        p_bias = pc_pool.tile([P, C], f32, tag="pbias")
        nc.tensor.matmul(out=p_bias[:], lhsT=ones_1p[:], rhs=b_vec[:], start=True, stop=True)
        bias_tile = const.tile([P, C], f32, tag="bias")
        nc.vector.tensor_copy(out=bias_tile[:], in_=p_bias[:])

        # --- main loop over superblocks --------------------------------------
        for s, m in enumerate(sb_meta):
            k0, k1, gw, gsb, woff = m["k0"], m["k1"], m["gw"], m["gsb"], m["woff"]
            kb = k1 - k0
            g_tile = gpool.tile([P, GSB_MAX * C], TBL_DT, tag="g")
            g3 = g_tile[:].rearrange("p (g c) -> p g c", c=C)
            icall = 0
            for w in range(NW):
                ngw = int(gw[w])
                if ngw == 0:
                    continue
                nc.gpsimd.dma_gather(
                    out_ap=g3[:, woff[w]:woff[w] + ngw, :],
                    in_ap=table[w * WROWS:(w + 1) * WROWS, :],
                    idxs_ap=idx_tiles[s][:, icall:icall + ngw * BLK // 16],
                    num_idxs=ngw * BLK,
                    num_idxs_reg=ngw * BLK,
                    elem_size=C,
                    queue_num=w,
                    single_packet=False,
                )
                icall += ngw * BLK // 16
            ostage = opool.tile([P, KB_MAX * C], f32, tag="ostage")
            boff = np.zeros(NW, np.int64)
            relcol = m["relbase"]
            for kk in range(kb):
                k = k0 + kk
                gb = int(gbtot[k])
                mask = mpool.tile([P, GB_MAX * BLK], TBL_DT, tag="mask")
                # one-hot mask via is_equal; rel is stored as duplicated
                # pairs so every operand's innermost dim is packed (DVE 2x).
                nc.vector.tensor_tensor(
                    out=mask[:, :gb * BLK].rearrange(
                        "p (g x y) -> p g x y", g=gb, y=2),
                    in0=iota_m[:, :gb * BLK].rearrange(
                        "p (g x y) -> p g x y", g=gb, y=2),
                    in1=rel_sb[:, relcol * 2:(relcol + gb) * 2].rearrange(
                        "p (g y) -> p g y", y=2).unsqueeze(2).to_broadcast(
                        [P, gb, BLK // 2, 2]),
                    op=mybir.AluOpType.is_equal,
                )
                pa = pa_pool.tile([C, BLK], f32, tag="pa")
                j = 0
                for w in range(NW):
                    for jj in range(int(g_uni[k, w])):
                        col = int(woff[w] + boff[w] + jj)
                        nc.tensor.matmul(
                            out=pa[:],
                            lhsT=g_tile[:, col * C:(col + 1) * C],
                            rhs=mask[:, j * BLK:(j + 1) * BLK],
                            start=(j == 0),
                            stop=(j == gb - 1),
                        )
                        j += 1
                agg = apool.tile([C, BLK], TBL_DT, tag="agg")
                nc.scalar.activation(agg[:], pa[:], mybir.ActivationFunctionType.Copy)
                pb = pb_pool.tile([BLK, C], f32, tag="pb")
                nc.tensor.matmul(out=pb[:], lhsT=agg[:], rhs=w16[:], start=True, stop=True)
                nc.vector.scalar_tensor_tensor(
                    out=ostage[:, kk * C:(kk + 1) * C],
                    in0=pb[:],
                    scalar=ivec_sb[:, k:k + 1],
                    in1=bias_tile[:],
                    op0=mybir.AluOpType.mult,
                    op1=mybir.AluOpType.add,
                )
                relcol += gb
                boff += g_uni[k]
            dram_view = out[k0 * BLK:k1 * BLK, :].rearrange(
                "(bb p) c -> p bb c", p=P
            )
            nc.sync.dma_start(
                out=dram_view,
                in_=ostage[:, :kb * C].rearrange("p (bb c) -> p bb c", bb=kb),
            )

    nc.compile()
    return nc


def _balance(dw):
    """Assign every dst id to a (block, lane) so that each (block, window)
    section's edge count fits the uniform capacity: 512 edges (4 groups),
    640 (5 groups) for the last block of each core.

    Greedy maximin placement by descending degree + pairwise swap repair.
    Returns members [NBLK, BLK] = dst id at (block, lane)."""
    caps = np.full((NBLK, NW), 4 * BLK, np.int64)
    caps[NB - 1::NB] = 5 * BLK
    order = np.argsort(-(dw.max(1) * 64 + dw.sum(1)), kind="stable")
    slack = caps.copy()
    slots = np.full(NBLK, BLK, np.int64)
    assign = np.empty(N_PAD, np.int64)
    BIG = -10**6
    for d in order:
        v = dw[d]
        score = (slack - v).min(axis=1)
        score[slots == 0] = BIG
        b = int(np.argmax(score))
        assign[d] = b
        slack[b] -= v
        slots[b] -= 1
    loads = caps - slack
    members = np.full((NBLK, BLK), -1, np.int64)
    cnt = np.zeros(NBLK, np.int64)
    for d in range(N_PAD):
        b = assign[d]
        members[b, cnt[b]] = d
        cnt[b] += 1
    rng = np.random.default_rng(0)
    it = 0
    while np.maximum(loads - caps, 0).sum() > 0 and it < 20000:
        it += 1
        ov = loads - caps
        b1, w1 = np.unravel_index(np.argmax(ov + rng.random(ov.shape) * 0.01), ov.shape)
        fixed = False
        for di in np.argsort(-dw[members[b1], w1])[:4]:
            d1 = members[b1, di]
            delta_all = dw[d1][None, None, :] - dw[members]
            nb2 = np.maximum(loads[:, None, :] + delta_all - caps[:, None, :], 0).sum(2)
            nb1 = np.maximum(loads[b1][None, None, :] - delta_all - caps[b1][None, None, :], 0).sum(2)
            cur = np.maximum(loads - caps, 0).sum(1)[:, None] \
                + np.maximum(loads[b1] - caps[b1], 0).sum()
            gain = cur - (nb1 + nb2)
            gain[b1, :] = -1
            b2, dj = np.unravel_index(np.argmax(gain), gain.shape)
            if gain[b2, dj] > 0:
                d2 = members[b2, dj]
                members[b1, di] = d2
                members[b2, dj] = d1
                loads[b1] += dw[d2] - dw[d1]
                loads[b2] += dw[d1] - dw[d2]
                fixed = True
                break
        if not fixed:
            break
    assert (loads <= caps).all(), "balance failed; residual overflow"
    return members


def _preprocess(feat, src, dst, W_mu, W_rho, bias_mu, bias_rho, W_eps, bias_eps):
    """Index-domain preprocessing + table pre-scaling. Returns (in_maps, key)."""
    src = np.asarray(src).astype(np.int64)
    dst = np.asarray(dst).astype(np.int64)
    feat = np.asarray(feat, dtype=np.float32)

    out_deg = np.bincount(src, minlength=N_NODES).astype(np.float32)
    o = 1.0 / np.sqrt(np.maximum(out_deg, 1.0))
    in_deg = np.bincount(dst, minlength=N_NODES)
    ivec_full = (1.0 / np.sqrt(np.maximum(in_deg, 1.0))).astype(np.float32)

    table = np.zeros((N_PAD, C), TBL_NP)
    table[:N_NODES] = (feat * o[:, None]).astype(TBL_NP)

    win = src // WROWS

    # --- balanced dst -> (block, lane) assignment ---------------------------
    dw = np.zeros((N_PAD, NW), np.int64)
    np.add.at(dw, (dst, win), 1)
    members = _balance(dw)                 # [NBLK, BLK] -> dst id
    blk_of = np.empty(N_PAD, np.int64)     # dst -> block
    lane_of = np.empty(N_PAD, np.int64)    # dst -> lane
    ar = np.arange(BLK)
    for b in range(NBLK):
        blk_of[members[b]] = b
        lane_of[members[b]] = ar

    # --- uniform structural layout ------------------------------------------
    g_uni = np.full((NB, NW), 4, np.int64)
    g_uni[NB - 1] = 5
    gbtot = g_uni.sum(axis=1)
    G_TOTAL = int(gbtot.sum())
    GB_MAX = int(gbtot.max())
    sbs = _plan_superblocks(g_uni)
    key = (tuple(g_uni.ravel().tolist()), tuple(sbs))

    col_base = np.zeros((NB, NW), np.int64)
    colbase = 0
    for (k0, k1) in sbs:
        gw = g_uni[k0:k1].sum(axis=0)
        woff = np.concatenate([[0], np.cumsum(gw)])[:NW]
        run = np.zeros(NW, np.int64)
        for k in range(k0, k1):
            col_base[k] = colbase + woff + run
            run += g_uni[k]
        colbase += int(gw.sum())
    assert colbase == G_TOTAL
    rel_base = np.concatenate([[0], np.cumsum(gbtot)])[:NB]
    wg_off = np.concatenate([np.zeros((NB, 1), np.int64),
                             np.cumsum(g_uni, axis=1)[:, :NW - 1]], axis=1)

    # --- per-edge slot computation ------------------------------------------
    gb_edge = blk_of[dst]
    core = gb_edge // NB
    k_edge = gb_edge % NB
    seckey = (core * NB + k_edge) * NW + win
    order = np.argsort(seckey, kind="stable")
    sk = seckey[order]
    ss = src[order]
    lane_s = lane_of[dst[order]]
    cnt = np.bincount(seckey, minlength=N_CORES * NB * NW)
    starts = np.zeros(N_CORES * NB * NW + 1, np.int64)
    np.cumsum(cnt, out=starts[1:])
    pos = np.arange(len(ss), dtype=np.int64) - starts[sk]

    kr = (sk // NW) % NB
    wr = sk % NW
    cr = sk // (NW * NB)
    slots_core = G_TOTAL * BLK
    slot = cr * slots_core + col_base[kr, wr] * BLK + pos
    idx_all = np.zeros(N_CORES * slots_core, np.int16)     # pad -> row 0
    idx_all[slot] = (ss - wr * WROWS).astype(np.int16)

    rel_slot = cr * slots_core + (rel_base[kr] + wg_off[kr, wr]) * BLK + pos
    rel_all = np.full(N_CORES * slots_core, -1.0, TBL_NP)
    rel_all[rel_slot] = lane_s.astype(TBL_NP)

    ivec_pad = np.ones(N_PAD, np.float32)
    ivec_pad[:N_NODES] = ivec_full

    iota_np = np.tile(np.arange(BLK, dtype=TBL_NP), GB_MAX)[None, :].repeat(P, 0)
    iota_np = np.ascontiguousarray(iota_np)

    cc = np.ascontiguousarray
    in_maps = []
    for c in range(N_CORES):
        idx_c = idx_all[c * slots_core:(c + 1) * slots_core]
        idx_tile = np.tile(idx_c.reshape(-1, 16).T, (8, 1))
        rel_c = rel_all[c * slots_core:(c + 1) * slots_core].reshape(G_TOTAL, P).T
        rel2 = np.repeat(rel_c[:, :, None], 2, axis=2).reshape(P, G_TOTAL * 2)
        iv = ivec_pad[members[c * NB:(c + 1) * NB]].T   # [P, NB]
        in_maps.append({
            "table": table,
            "iota": iota_np,
            "idx": cc(idx_tile),
            "rel": cc(rel2),
            "ivec": cc(iv.astype(np.float32)),
            "w_mu": np.asarray(W_mu, np.float32),
            "w_rho": np.asarray(W_rho, np.float32),
            "w_eps": np.asarray(W_eps, np.float32),
            "b_mu": np.asarray(bias_mu, np.float32).reshape(1, C),
            "b_rho": np.asarray(bias_rho, np.float32).reshape(1, C),
            "b_eps": np.asarray(bias_eps, np.float32).reshape(1, C),
        })
    in_maps[0]["_members"] = members
    return in_maps, key


def kernel(**inputs) -> np.ndarray:
    in_maps, key = _preprocess(**inputs)
    members = in_maps[0].pop("_members")
    if key not in _CACHE:
        _CACHE[key] = _build_program(key)
    nc = _CACHE[key]
    res = run_bass_kernel_spmd(nc, in_maps, core_ids=list(range(N_CORES)))
    out_full = np.empty((N_PAD, C), np.float32)
    for c in range(N_CORES):
        rows = members[c * NB:(c + 1) * NB].ravel()
        out_full[rows] = res.results[c]["out"]
    return out_full[:N_NODES]


# revision 7
# speedup vs baseline: 1.3040x; 1.0128x over previous
"""Bass/Trainium2 kernel for nn_BBBGraphConv (Bayesian GraphConv, DGL norm='both').

Computation (reference):
    W    = W_mu + W_eps * softplus(W_rho)
    bias = bias_mu + bias_eps * softplus(bias_rho)
    o    = clip(out_deg, 1)^-0.5 ; i = clip(in_deg, 1)^-0.5
    out  = segsum_dst((feat * o)[src]) @ W * i[:, None] + bias

Distribution: dst nodes are assigned to (core, block, lane) by a host-side
load balancer so that every (block, src-window) section holds at most
512 edges (640 for one designated overflow block per core).  This gives a
single uniform SPMD program: 97 blocks x 4 groups per window plus one block
x 5 groups per window per core, with ~0.6% gather padding.  Each of the 8
cores owns 98 blocks of 128 dst nodes and computes their output rows
exclusively (no collective needed).  The pre-scaled fp16 feature table is
replicated; each core gathers the source rows of its edges with gpsimd
dma_gather (the src id space is split into 4 windows of 25088 rows for the
int16 index format), reduces them per destination with TensorE one-hot-mask
matmuls, projects through W in fp16, applies dst-side norm + bias, and
writes its slice.  The host un-permutes the rows afterwards.

The gather descriptor supply (Q7 SWDGE descriptor generation, ~7.3ns per
index per queue across 4 queues) is the end-to-end bottleneck, so the
design minimizes the gathered index count above all else.

The one-hot masks are built with is_equal on operands whose innermost dims
are packed (rel duplicated in pairs, viewed [p,g,64,2]) to hit the DVE
2x_1p fast path; the projection runs in fp16 (1 cycle/row on the PE).
"""

import numpy as np
from contextlib import ExitStack

import concourse.bass as bass
import concourse.bacc as bacc
import concourse.tile as tile
from concourse import mybir
from concourse.bass_utils import run_bass_kernel_spmd

# Problem constants (hardcoded per the harness contract)
N_NODES = 100_000
N_EDGES = 1_600_000
C = 128          # in_ch == out_ch
P = 128          # partitions
N_CORES = 8
BLK = 128        # dst nodes per block
NB = 98          # blocks per core
D_CORE = NB * BLK          # 12544 dst rows per core
N_PAD = N_CORES * D_CORE   # 100352
NBLK = N_CORES * NB        # 784 blocks total

NW = 4           # src windows (dma_gather indices are int16)
WROWS = N_PAD // NW        # 25088 rows per window

CALL_CAP = 35    # max groups per dma_gather call (num_idxs <= 4480)
OVF_K = 49       # structural block with 5 groups/window (overflow capacity)

TBL_DT = mybir.dt.float16
TBL_NP = np.float16

_CACHE: dict = {}


def _plan_superblocks(g_uni):
    """Pack blocks into superblocks: small at both ends (fast pipeline rampup
    and short tail), 8-block bodies, respecting the per-window call cap."""
    sizes = [4, 4] + [8] * 10 + [4, 4, 2]
    assert sum(sizes) == NB
    sbs = []
    k0 = 0
    for s in sizes:
        assert g_uni[k0:k0 + s].sum(axis=0).max() <= CALL_CAP
        sbs.append((k0, k0 + s))
        k0 += s
    return sbs


def _build_program(key):
    """Build the SPMD Bass program (one graph, runs on all 8 cores)."""
    g_uni_flat, sbs = key
    g_uni = np.asarray(g_uni_flat, np.int64).reshape(NB, NW)
    gbtot = g_uni.sum(axis=1)            # mask groups per block
    GB_MAX = int(gbtot.max())
    G_TOTAL = int(gbtot.sum())           # total groups per core
    idx_f_total = G_TOTAL * BLK // 16    # int16 idx values per partition row
    f32 = mybir.dt.float32

    sb_meta = []
    colbase = 0
    idxbase = 0
    relbase = 0
    for (k0, k1) in sbs:
        gw = g_uni[k0:k1].sum(axis=0)
        gsb = int(gw.sum())
        woff = np.concatenate([[0], np.cumsum(gw)])[:NW]
        sb_meta.append(dict(k0=k0, k1=k1, gw=gw, gsb=gsb, woff=woff,
                            colbase=colbase, idxbase=idxbase, relbase=relbase))
        colbase += gsb
        idxbase += gsb * BLK // 16
        relbase += int(gbtot[k0:k1].sum())
    assert colbase == G_TOTAL

    nc = bacc.Bacc("TRN2", target_bir_lowering=False, debug=False, num_swdge_queues=4)

    table = nc.dram_tensor("table", [N_PAD, C], TBL_DT, kind="ExternalInput").ap()
    idx_t = nc.dram_tensor("idx", [P, idx_f_total], mybir.dt.int16,
                           kind="ExternalInput").ap()
    rel_t = nc.dram_tensor("rel", [P, G_TOTAL * 2], TBL_DT, kind="ExternalInput").ap()
    iota_t = nc.dram_tensor("iota", [P, GB_MAX * BLK], TBL_DT, kind="ExternalInput").ap()
    ivec_t = nc.dram_tensor("ivec", [P, NB], f32, kind="ExternalInput").ap()
    w_mu = nc.dram_tensor("w_mu", [C, C], f32, kind="ExternalInput").ap()
    w_rho = nc.dram_tensor("w_rho", [C, C], f32, kind="ExternalInput").ap()
    w_eps = nc.dram_tensor("w_eps", [C, C], f32, kind="ExternalInput").ap()
    b_mu = nc.dram_tensor("b_mu", [1, C], f32, kind="ExternalInput").ap()
    b_rho = nc.dram_tensor("b_rho", [1, C], f32, kind="ExternalInput").ap()
    b_eps = nc.dram_tensor("b_eps", [1, C], f32, kind="ExternalInput").ap()
    out = nc.dram_tensor("out", [D_CORE, C], f32, kind="ExternalOutput").ap()

    GSB_MAX = max(m["gsb"] for m in sb_meta)
    KB_MAX = max(m["k1"] - m["k0"] for m in sb_meta)

    with tile.TileContext(nc) as tc, ExitStack() as ctx:
        const = ctx.enter_context(tc.tile_pool(name="const", bufs=1))
        gpool = ctx.enter_context(tc.tile_pool(name="gather", bufs=3))
        mpool = ctx.enter_context(tc.tile_pool(name="mask", bufs=5))
        apool = ctx.enter_context(tc.tile_pool(name="aggf", bufs=3))
        opool = ctx.enter_context(tc.tile_pool(name="ostage", bufs=3))
        pa_pool = ctx.enter_context(tc.tile_pool(name="pa", bufs=3, space="PSUM"))
        pb_pool = ctx.enter_context(tc.tile_pool(name="pb", bufs=2, space="PSUM"))
        pc_pool = ctx.enter_context(tc.tile_pool(name="pc", bufs=1, space="PSUM"))

        # --- resident inputs -------------------------------------------------
        idx_tiles = []
        for s, m in enumerate(sb_meta):
            n = m["gsb"] * BLK // 16
            t = const.tile([P, n], mybir.dt.int16, tag=f"idx{s}")
            nc.sync.dma_start(out=t[:], in_=idx_t[:, m["idxbase"]:m["idxbase"] + n])
            idx_tiles.append(t)
        rel_sb = const.tile([P, G_TOTAL * 2], TBL_DT, tag="rel")
        nc.sync.dma_start(out=rel_sb[:], in_=rel_t[:])
        ivec_sb = const.tile([P, NB], f32, tag="ivec")
        nc.sync.dma_start(out=ivec_sb[:], in_=ivec_t[:])
        iota_m = const.tile([P, GB_MAX * BLK], TBL_DT, tag="iotam")
        nc.sync.dma_start(out=iota_m[:], in_=iota_t[:])

        # --- W = W_mu + W_eps * softplus(W_rho), cast to fp16 ---------------
        wmu_sb = const.tile([C, C], f32, tag="wmu")
        nc.sync.dma_start(out=wmu_sb[:], in_=w_mu[:])
        wrho_sb = const.tile([C, C], f32, tag="wrho")
        nc.sync.dma_start(out=wrho_sb[:], in_=w_rho[:])
        weps_sb = const.tile([C, C], f32, tag="weps")
        nc.sync.dma_start(out=weps_sb[:], in_=w_eps[:])
        w_sp = const.tile([C, C], f32, tag="wsp")
        nc.scalar.activation(w_sp[:], wrho_sb[:], mybir.ActivationFunctionType.Exp)
        nc.scalar.activation(w_sp[:], w_sp[:], mybir.ActivationFunctionType.Ln, bias=1.0)
        w_f32 = const.tile([C, C], f32, tag="wf32")
        nc.vector.tensor_tensor(out=w_f32[:], in0=weps_sb[:], in1=w_sp[:], op=mybir.AluOpType.mult)
        nc.vector.tensor_tensor(out=w_f32[:], in0=w_f32[:], in1=wmu_sb[:], op=mybir.AluOpType.add)
        w16 = const.tile([C, C], TBL_DT, tag="w16")
        nc.scalar.activation(w16[:], w_f32[:], mybir.ActivationFunctionType.Copy)

        # --- bias tile [P, C]: every partition row holds the bias vector -----
        bmu_sb = const.tile([1, C], f32, tag="bmu")
        nc.sync.dma_start(out=bmu_sb[:], in_=b_mu[:])
        brho_sb = const.tile([1, C], f32, tag="brho")
        nc.sync.dma_start(out=brho_sb[:], in_=b_rho[:])
        beps_sb = const.tile([1, C], f32, tag="beps")
        nc.sync.dma_start(out=beps_sb[:], in_=b_eps[:])
        b_sp = const.tile([1, C], f32, tag="bsp")
        nc.scalar.activation(b_sp[:], brho_sb[:], mybir.ActivationFunctionType.Exp)
        nc.scalar.activation(b_sp[:], b_sp[:], mybir.ActivationFunctionType.Ln, bias=1.0)
        b_vec = const.tile([1, C], f32, tag="bvec")
        nc.vector.tensor_tensor(out=b_vec[:], in0=beps_sb[:], in1=b_sp[:], op=mybir.AluOpType.mult)
        nc.vector.tensor_tensor(out=b_vec[:], in0=b_vec[:], in1=bmu_sb[:], op=mybir.AluOpType.add)
        ones_1p = const.tile([1, C], f32, tag="ones")
        nc.vector.memset(ones_1p[:], 1.0)
        p_bias = pc_pool.tile([P, C], f32, tag="pbias")
        nc.tensor.matmul(out=p_bias[:], lhsT=ones_1p[:], rhs=b_vec[:], start=True, stop=True)
        bias_tile = const.tile([P, C], f32, tag="bias")
        nc.vector.tensor_copy(out=bias_tile[:], in_=p_bias[:])

        # --- main loop over superblocks --------------------------------------
        for s, m in enumerate(sb_meta):
            k0, k1, gw, gsb, woff = m["k0"], m["k1"], m["gw"], m["gsb"], m["woff"]
            kb = k1 - k0
            g_tile = gpool.tile([P, GSB_MAX * C], TBL_DT, tag="g")
            g3 = g_tile[:].rearrange("p (g c) -> p g c", c=C)
            icall = 0
            for w in range(NW):
                ngw = int(gw[w])
                if ngw == 0:
                    continue
                nc.gpsimd.dma_gather(
                    out_ap=g3[:, woff[w]:woff[w] + ngw, :],
                    in_ap=table[w * WROWS:(w + 1) * WROWS, :],
                    idxs_ap=idx_tiles[s][:, icall:icall + ngw * BLK // 16],
                    num_idxs=ngw * BLK,
                    num_idxs_reg=ngw * BLK,
                    elem_size=C,
                    queue_num=w,
                    single_packet=False,
                )
                icall += ngw * BLK // 16
            ostage = opool.tile([P, KB_MAX * C], f32, tag="ostage")
            boff = np.zeros(NW, np.int64)
            relcol = m["relbase"]
            for kk in range(kb):
                k = k0 + kk
                gb = int(gbtot[k])
                mask = mpool.tile([P, GB_MAX * BLK], TBL_DT, tag="mask")
                # one-hot mask via is_equal; rel is stored as duplicated
                # pairs so every operand's innermost dim is packed (DVE 2x).
                nc.vector.tensor_tensor(
                    out=mask[:, :gb * BLK].rearrange(
                        "p (g x y) -> p g x y", g=gb, y=2),
                    in0=iota_m[:, :gb * BLK].rearrange(
                        "p (g x y) -> p g x y", g=gb, y=2),
                    in1=rel_sb[:, relcol * 2:(relcol + gb) * 2].rearrange(
                        "p (g y) -> p g y", y=2).unsqueeze(2).to_broadcast(
                        [P, gb, BLK // 2, 2]),
                    op=mybir.AluOpType.is_equal,
                )
                pa = pa_pool.tile([C, BLK], f32, tag="pa")
                j = 0
                for w in range(NW):
                    for jj in range(int(g_uni[k, w])):
                        col = int(woff[w] + boff[w] + jj)
                        nc.tensor.matmul(
                            out=pa[:],
                            lhsT=g_tile[:, col * C:(col + 1) * C],
                            rhs=mask[:, j * BLK:(j + 1) * BLK],
                            start=(j == 0),
                            stop=(j == gb - 1),
                        )
                        j += 1
                agg = apool.tile([C, BLK], TBL_DT, tag="agg")
                nc.scalar.activation(agg[:], pa[:], mybir.ActivationFunctionType.Copy)
                pb = pb_pool.tile([BLK, C], f32, tag="pb")
                nc.tensor.matmul(out=pb[:], lhsT=agg[:], rhs=w16[:], start=True, stop=True)
                nc.vector.scalar_tensor_tensor(
                    out=ostage[:, kk * C:(kk + 1) * C],
                    in0=pb[:],
                    scalar=ivec_sb[:, k:k + 1],
                    in1=bias_tile[:],
                    op0=mybir.AluOpType.mult,
                    op1=mybir.AluOpType.add,
                )
                relcol += gb
                boff += g_uni[k]
            dram_view = out[k0 * BLK:k1 * BLK, :].rearrange(
                "(bb p) c -> p bb c", p=P
            )
            nc.sync.dma_start(
                out=dram_view,
                in_=ostage[:, :kb * C].rearrange("p (bb c) -> p bb c", bb=kb),
            )

    nc.compile()
    return nc


def _balance_caps(dw, caps):
    """Greedy maximin placement by descending degree + pairwise swap repair.
    Returns (members [NBLK, BLK], ok)."""
    order = np.argsort(-(dw.max(1) * 64 + dw.sum(1)), kind="stable")
    slack = caps.copy()
    slots = np.full(NBLK, BLK, np.int64)
    assign = np.empty(N_PAD, np.int64)
    BIG = -10**6
    for d in order:
        v = dw[d]
        score = (slack - v).min(axis=1)
        score[slots == 0] = BIG
        b = int(np.argmax(score))
        assign[d] = b
        slack[b] -= v
        slots[b] -= 1
    loads = caps - slack
    members = np.full((NBLK, BLK), -1, np.int64)
    cnt = np.zeros(NBLK, np.int64)
    for d in range(N_PAD):
        b = assign[d]
        members[b, cnt[b]] = d
        cnt[b] += 1
    rng = np.random.default_rng(0)
    it = 0
    while np.maximum(loads - caps, 0).sum() > 0 and it < 20000:
        it += 1
        ov = loads - caps
        b1, w1 = np.unravel_index(np.argmax(ov + rng.random(ov.shape) * 0.01), ov.shape)
        fixed = False
        for di in np.argsort(-dw[members[b1], w1])[:4]:
            d1 = members[b1, di]
            delta_all = dw[d1][None, None, :] - dw[members]
            nb2 = np.maximum(loads[:, None, :] + delta_all - caps[:, None, :], 0).sum(2)
            nb1 = np.maximum(loads[b1][None, None, :] - delta_all - caps[b1][None, None, :], 0).sum(2)
            cur = np.maximum(loads - caps, 0).sum(1)[:, None] \
                + np.maximum(loads[b1] - caps[b1], 0).sum()
            gain = cur - (nb1 + nb2)
            gain[b1, :] = -1
            b2, dj = np.unravel_index(np.argmax(gain), gain.shape)
            if gain[b2, dj] > 0:
                d2 = members[b2, dj]
                members[b1, di] = d2
                members[b2, dj] = d1
                loads[b1] += dw[d2] - dw[d1]
                loads[b2] += dw[d1] - dw[d2]
                fixed = True
                break
        if not fixed:
            break
    return members, bool((loads <= caps).all())


def _balance(dw):
    """Assign every dst id to a (block, lane) so that each (block, window)
    section's edge count fits a uniform per-block capacity.  Starts with one
    5-group overflow block per core and widens capacity if repair fails.
    Returns (members, g_uni)."""
    for extra in range(4):
        caps = np.full((NBLK, NW), 4 * BLK, np.int64)
        g_uni = np.full((NB, NW), 4, np.int64)
        for j in range(1 + extra):
            k = (OVF_K + j * 7) % NB
            caps[k::NB] = 5 * BLK
            g_uni[k] = 5
        members, ok = _balance_caps(dw, caps)
        if ok:
            return members, g_uni
    raise AssertionError("balance failed; residual overflow")


def _preprocess(feat, src, dst, W_mu, W_rho, bias_mu, bias_rho, W_eps, bias_eps):
    """Index-domain preprocessing + table pre-scaling. Returns (in_maps, key)."""
    src = np.asarray(src).astype(np.int64)
    dst = np.asarray(dst).astype(np.int64)
    feat = np.asarray(feat, dtype=np.float32)

    out_deg = np.bincount(src, minlength=N_NODES).astype(np.float32)
    o = 1.0 / np.sqrt(np.maximum(out_deg, 1.0))
    in_deg = np.bincount(dst, minlength=N_NODES)
    ivec_full = (1.0 / np.sqrt(np.maximum(in_deg, 1.0))).astype(np.float32)

    table = np.zeros((N_PAD, C), TBL_NP)
    table[:N_NODES] = (feat * o[:, None]).astype(TBL_NP)

    win = src // WROWS

    # --- balanced dst -> (block, lane) assignment ---------------------------
    dw = np.zeros((N_PAD, NW), np.int64)
    np.add.at(dw, (dst, win), 1)
    members, g_uni = _balance(dw)          # [NBLK, BLK] -> dst id
    blk_of = np.empty(N_PAD, np.int64)     # dst -> block
    lane_of = np.empty(N_PAD, np.int64)    # dst -> lane
    ar = np.arange(BLK)
    for b in range(NBLK):
        blk_of[members[b]] = b
        lane_of[members[b]] = ar

    # --- uniform structural layout ------------------------------------------
    gbtot = g_uni.sum(axis=1)
    G_TOTAL = int(gbtot.sum())
    GB_MAX = int(gbtot.max())
    sbs = _plan_superblocks(g_uni)
    key = (tuple(g_uni.ravel().tolist()), tuple(sbs))

    col_base = np.zeros((NB, NW), np.int64)
    colbase = 0
    for (k0, k1) in sbs:
        gw = g_uni[k0:k1].sum(axis=0)
        woff = np.concatenate([[0], np.cumsum(gw)])[:NW]
        run = np.zeros(NW, np.int64)
        for k in range(k0, k1):
            col_base[k] = colbase + woff + run
            run += g_uni[k]
        colbase += int(gw.sum())
    assert colbase == G_TOTAL
    rel_base = np.concatenate([[0], np.cumsum(gbtot)])[:NB]
    wg_off = np.concatenate([np.zeros((NB, 1), np.int64),
                             np.cumsum(g_uni, axis=1)[:, :NW - 1]], axis=1)

    # --- per-edge slot computation ------------------------------------------
    gb_edge = blk_of[dst]
    core = gb_edge // NB
    k_edge = gb_edge % NB
    seckey = (core * NB + k_edge) * NW + win
    order = np.argsort(seckey, kind="stable")
    sk = seckey[order]
    ss = src[order]
    lane_s = lane_of[dst[order]]
    cnt = np.bincount(seckey, minlength=N_CORES * NB * NW)
    starts = np.zeros(N_CORES * NB * NW + 1, np.int64)
    np.cumsum(cnt, out=starts[1:])
    pos = np.arange(len(ss), dtype=np.int64) - starts[sk]

    kr = (sk // NW) % NB
    wr = sk % NW
    cr = sk // (NW * NB)
    slots_core = G_TOTAL * BLK
    slot = cr * slots_core + col_base[kr, wr] * BLK + pos
    idx_all = np.zeros(N_CORES * slots_core, np.int16)     # pad -> row 0
    idx_all[slot] = (ss - wr * WROWS).astype(np.int16)

    rel_slot = cr * slots_core + (rel_base[kr] + wg_off[kr, wr]) * BLK + pos
    rel_all = np.full(N_CORES * slots_core, -1.0, TBL_NP)
    rel_all[rel_slot] = lane_s.astype(TBL_NP)

    ivec_pad = np.ones(N_PAD, np.float32)
    ivec_pad[:N_NODES] = ivec_full

    iota_np = np.tile(np.arange(BLK, dtype=TBL_NP), GB_MAX)[None, :].repeat(P, 0)
    iota_np = np.ascontiguousarray(iota_np)

    cc = np.ascontiguousarray
    in_maps = []
    for c in range(N_CORES):
        idx_c = idx_all[c * slots_core:(c + 1) * slots_core]
        idx_tile = np.tile(idx_c.reshape(-1, 16).T, (8, 1))
        rel_c = rel_all[c * slots_core:(c + 1) * slots_core].reshape(G_TOTAL, P).T
        rel2 = np.repeat(rel_c[:, :, None], 2, axis=2).reshape(P, G_TOTAL * 2)
        iv = ivec_pad[members[c * NB:(c + 1) * NB]].T   # [P, NB]
        in_maps.append({
            "table": table,
            "iota": iota_np,
            "idx": cc(idx_tile),
            "rel": cc(rel2),
            "ivec": cc(iv.astype(np.float32)),
            "w_mu": np.asarray(W_mu, np.float32),
            "w_rho": np.asarray(W_rho, np.float32),
            "w_eps": np.asarray(W_eps, np.float32),
            "b_mu": np.asarray(bias_mu, np.float32).reshape(1, C),
            "b_rho": np.asarray(bias_rho, np.float32).reshape(1, C),
            "b_eps": np.asarray(bias_eps, np.float32).reshape(1, C),
        })
    in_maps[0]["_members"] = members
    return in_maps, key


def kernel(**inputs) -> np.ndarray:
    in_maps, key = _preprocess(**inputs)
    members = in_maps[0].pop("_members")
    if key not in _CACHE:
        _CACHE[key] = _build_program(key)
    nc = _CACHE[key]
    res = run_bass_kernel_spmd(nc, in_maps, core_ids=list(range(N_CORES)))
    out_full = np.empty((N_PAD, C), np.float32)
    for c in range(N_CORES):
        rows = members[c * NB:(c + 1) * NB].ravel()
        out_full[rows] = res.results[c]["out"]
    return out_full[:N_NODES]


# revision 8
# speedup vs baseline: 1.3067x; 1.0021x over previous
"""Bass/Trainium2 kernel for nn_BBBGraphConv (Bayesian GraphConv, DGL norm='both').

Computation (reference):
    W    = W_mu + W_eps * softplus(W_rho)
    bias = bias_mu + bias_eps * softplus(bias_rho)
    o    = clip(out_deg, 1)^-0.5 ; i = clip(in_deg, 1)^-0.5
    out  = segsum_dst((feat * o)[src]) @ W * i[:, None] + bias

Distribution: dst nodes are assigned to (core, block, lane) by a host-side
load balancer so that every (block, src-window) section holds at most
512 edges (640 for one designated overflow block per core).  This gives a
single uniform SPMD program: 97 blocks x 4 groups per window plus one block
x 5 groups per window per core, with ~0.6% gather padding.  Each of the 8
cores owns 98 blocks of 128 dst nodes and computes their output rows
exclusively (no collective needed).  The pre-scaled fp16 feature table is
replicated; each core gathers the source rows of its edges with gpsimd
dma_gather (the src id space is split into 4 windows of 25088 rows for the
int16 index format), reduces them per destination with TensorE one-hot-mask
matmuls, projects through W in fp16, applies dst-side norm + bias, and
writes its slice.  The host un-permutes the rows afterwards.

The gather descriptor supply (Q7 SWDGE descriptor generation, ~7.3ns per
index per queue across 4 queues) is the end-to-end bottleneck, so the
design minimizes the gathered index count above all else.

The one-hot masks are built with is_equal on operands whose innermost dims
are packed (rel duplicated in pairs, viewed [p,g,64,2]) to hit the DVE
2x_1p fast path; the projection runs in fp16 (1 cycle/row on the PE).
"""

import numpy as np
from contextlib import ExitStack

import concourse.bass as bass
import concourse.bacc as bacc
import concourse.tile as tile
from concourse import mybir
from concourse.bass_utils import run_bass_kernel_spmd

# Problem constants (hardcoded per the harness contract)
N_NODES = 100_000
N_EDGES = 1_600_000
C = 128          # in_ch == out_ch
P = 128          # partitions
N_CORES = 8
BLK = 128        # dst nodes per block
NB = 98          # blocks per core
D_CORE = NB * BLK          # 12544 dst rows per core
N_PAD = N_CORES * D_CORE   # 100352
NBLK = N_CORES * NB        # 784 blocks total

NW = 4           # src windows (dma_gather indices are int16)
WROWS = N_PAD // NW        # 25088 rows per window

CALL_CAP = 35    # max groups per dma_gather call (num_idxs <= 4480)
OVF_K = 49       # structural block with 5 groups/window (overflow capacity)

TBL_DT = mybir.dt.float16
TBL_NP = np.float16

_CACHE: dict = {}


def _plan_superblocks(g_uni):
    """Pack blocks into superblocks: small at both ends (fast pipeline rampup
    and short tail), 8-block bodies, respecting the per-window call cap."""
    sizes = [4, 4] + [8] * 10 + [4, 3, 2, 1]
    assert sum(sizes) == NB
    sbs = []
    k0 = 0
    for s in sizes:
        assert g_uni[k0:k0 + s].sum(axis=0).max() <= CALL_CAP
        sbs.append((k0, k0 + s))
        k0 += s
    return sbs


def _build_program(key):
    """Build the SPMD Bass program (one graph, runs on all 8 cores)."""
    g_uni_flat, sbs = key
    g_uni = np.asarray(g_uni_flat, np.int64).reshape(NB, NW)
    gbtot = g_uni.sum(axis=1)            # mask groups per block
    GB_MAX = int(gbtot.max())
    G_TOTAL = int(gbtot.sum())           # total groups per core
    idx_f_total = G_TOTAL * BLK // 16    # int16 idx values per partition row
    f32 = mybir.dt.float32

    sb_meta = []
    colbase = 0
    idxbase = 0
    relbase = 0
    for (k0, k1) in sbs:
        gw = g_uni[k0:k1].sum(axis=0)
        gsb = int(gw.sum())
        woff = np.concatenate([[0], np.cumsum(gw)])[:NW]
        sb_meta.append(dict(k0=k0, k1=k1, gw=gw, gsb=gsb, woff=woff,
                            colbase=colbase, idxbase=idxbase, relbase=relbase))
        colbase += gsb
        idxbase += gsb * BLK // 16
        relbase += int(gbtot[k0:k1].sum())
    assert colbase == G_TOTAL

    nc = bacc.Bacc("TRN2", target_bir_lowering=False, debug=False, num_swdge_queues=4)

    table = nc.dram_tensor("table", [N_PAD, C], TBL_DT, kind="ExternalInput").ap()
    idx_t = nc.dram_tensor("idx", [P, idx_f_total], mybir.dt.int16,
                           kind="ExternalInput").ap()
    rel_t = nc.dram_tensor("rel", [P, G_TOTAL * 2], TBL_DT, kind="ExternalInput").ap()
    iota_t = nc.dram_tensor("iota", [P, GB_MAX * BLK], TBL_DT, kind="ExternalInput").ap()
    ivec_t = nc.dram_tensor("ivec", [P, NB], f32, kind="ExternalInput").ap()
    w_mu = nc.dram_tensor("w_mu", [C, C], f32, kind="ExternalInput").ap()
    w_rho = nc.dram_tensor("w_rho", [C, C], f32, kind="ExternalInput").ap()
    w_eps = nc.dram_tensor("w_eps", [C, C], f32, kind="ExternalInput").ap()
    b_mu = nc.dram_tensor("b_mu", [1, C], f32, kind="ExternalInput").ap()
    b_rho = nc.dram_tensor("b_rho", [1, C], f32, kind="ExternalInput").ap()
    b_eps = nc.dram_tensor("b_eps", [1, C], f32, kind="ExternalInput").ap()
    out = nc.dram_tensor("out", [D_CORE, C], f32, kind="ExternalOutput").ap()

    GSB_MAX = max(m["gsb"] for m in sb_meta)
    KB_MAX = max(m["k1"] - m["k0"] for m in sb_meta)

    with tile.TileContext(nc) as tc, ExitStack() as ctx:
        const = ctx.enter_context(tc.tile_pool(name="const", bufs=1))
        gpool = ctx.enter_context(tc.tile_pool(name="gather", bufs=3))
        mpool = ctx.enter_context(tc.tile_pool(name="mask", bufs=5))
        apool = ctx.enter_context(tc.tile_pool(name="aggf", bufs=3))
        opool = ctx.enter_context(tc.tile_pool(name="ostage", bufs=3))
        pa_pool = ctx.enter_context(tc.tile_pool(name="pa", bufs=3, space="PSUM"))
        pb_pool = ctx.enter_context(tc.tile_pool(name="pb", bufs=2, space="PSUM"))
        pc_pool = ctx.enter_context(tc.tile_pool(name="pc", bufs=1, space="PSUM"))

        # --- gather issue helper --------------------------------------------
        g_tiles = {}

        def emit_gathers(s, m, idx_tile):
            g_tile = gpool.tile([P, GSB_MAX * C], TBL_DT, tag="g")
            g_tiles[s] = g_tile
            g3 = g_tile[:].rearrange("p (g c) -> p g c", c=C)
            icall = 0
            for w in range(NW):
                ngw = int(m["gw"][w])
                if ngw == 0:
                    continue
                nc.gpsimd.dma_gather(
                    out_ap=g3[:, m["woff"][w]:m["woff"][w] + ngw, :],
                    in_ap=table[w * WROWS:(w + 1) * WROWS, :],
                    idxs_ap=idx_tile[:, icall:icall + ngw * BLK // 16],
                    num_idxs=ngw * BLK,
                    num_idxs_reg=ngw * BLK,
                    elem_size=C,
                    queue_num=w,
                    single_packet=False,
                )
                icall += ngw * BLK // 16

        # --- resident inputs -------------------------------------------------
        # the first two superblocks' idx tiles and gather calls are emitted
        # before everything else so the first desc-gen does not wait on the
        # batched DMA semaphore of the remaining constant uploads
        idx_tiles = {}
        N_EARLY = 2
        for s in range(N_EARLY):
            m = sb_meta[s]
            n = m["gsb"] * BLK // 16
            t = const.tile([P, n], mybir.dt.int16, tag=f"idx{s}")
            nc.sync.dma_start(out=t[:], in_=idx_t[:, m["idxbase"]:m["idxbase"] + n])
            idx_tiles[s] = t
            emit_gathers(s, m, t)
        for s in range(N_EARLY, len(sb_meta)):
            m = sb_meta[s]
            n = m["gsb"] * BLK // 16
            t = const.tile([P, n], mybir.dt.int16, tag=f"idx{s}")
            nc.sync.dma_start(out=t[:], in_=idx_t[:, m["idxbase"]:m["idxbase"] + n])
            idx_tiles[s] = t
        rel_sb = const.tile([P, G_TOTAL * 2], TBL_DT, tag="rel")
        nc.sync.dma_start(out=rel_sb[:], in_=rel_t[:])
        ivec_sb = const.tile([P, NB], f32, tag="ivec")
        nc.sync.dma_start(out=ivec_sb[:], in_=ivec_t[:])
        iota_m = const.tile([P, GB_MAX * BLK], TBL_DT, tag="iotam")
        nc.sync.dma_start(out=iota_m[:], in_=iota_t[:])

        # --- W = W_mu + W_eps * softplus(W_rho), cast to fp16 ---------------
        wmu_sb = const.tile([C, C], f32, tag="wmu")
        nc.sync.dma_start(out=wmu_sb[:], in_=w_mu[:])
        wrho_sb = const.tile([C, C], f32, tag="wrho")
        nc.sync.dma_start(out=wrho_sb[:], in_=w_rho[:])
        weps_sb = const.tile([C, C], f32, tag="weps")
        nc.sync.dma_start(out=weps_sb[:], in_=w_eps[:])
        w_sp = const.tile([C, C], f32, tag="wsp")
        nc.scalar.activation(w_sp[:], wrho_sb[:], mybir.ActivationFunctionType.Exp)
        nc.scalar.activation(w_sp[:], w_sp[:], mybir.ActivationFunctionType.Ln, bias=1.0)
        w_f32 = const.tile([C, C], f32, tag="wf32")
        nc.vector.tensor_tensor(out=w_f32[:], in0=weps_sb[:], in1=w_sp[:], op=mybir.AluOpType.mult)
        nc.vector.tensor_tensor(out=w_f32[:], in0=w_f32[:], in1=wmu_sb[:], op=mybir.AluOpType.add)
        w16 = const.tile([C, C], TBL_DT, tag="w16")
        nc.scalar.activation(w16[:], w_f32[:], mybir.ActivationFunctionType.Copy)

        # --- bias tile [P, C]: every partition row holds the bias vector -----
        bmu_sb = const.tile([1, C], f32, tag="bmu")
        nc.sync.dma_start(out=bmu_sb[:], in_=b_mu[:])
        brho_sb = const.tile([1, C], f32, tag="brho")
        nc.sync.dma_start(out=brho_sb[:], in_=b_rho[:])
        beps_sb = const.tile([1, C], f32, tag="beps")
        nc.sync.dma_start(out=beps_sb[:], in_=b_eps[:])
        b_sp = const.tile([1, C], f32, tag="bsp")
        nc.scalar.activation(b_sp[:], brho_sb[:], mybir.ActivationFunctionType.Exp)
        nc.scalar.activation(b_sp[:], b_sp[:], mybir.ActivationFunctionType.Ln, bias=1.0)
        b_vec = const.tile([1, C], f32, tag="bvec")
        nc.vector.tensor_tensor(out=b_vec[:], in0=beps_sb[:], in1=b_sp[:], op=mybir.AluOpType.mult)
        nc.vector.tensor_tensor(out=b_vec[:], in0=b_vec[:], in1=bmu_sb[:], op=mybir.AluOpType.add)
        ones_1p = const.tile([1, C], f32, tag="ones")
        nc.vector.memset(ones_1p[:], 1.0)
        p_bias = pc_pool.tile([P, C], f32, tag="pbias")
        nc.tensor.matmul(out=p_bias[:], lhsT=ones_1p[:], rhs=b_vec[:], start=True, stop=True)
        bias_tile = const.tile([P, C], f32, tag="bias")
        nc.vector.tensor_copy(out=bias_tile[:], in_=p_bias[:])

        # --- main loop over superblocks --------------------------------------
        for s, m in enumerate(sb_meta):
            k0, k1, gw, gsb, woff = m["k0"], m["k1"], m["gw"], m["gsb"], m["woff"]
            kb = k1 - k0
            if s >= N_EARLY:
                emit_gathers(s, m, idx_tiles[s])
            g_tile = g_tiles[s]
            ostage = opool.tile([P, KB_MAX * C], f32, tag="ostage")
            boff = np.zeros(NW, np.int64)
            relcol = m["relbase"]
            for kk in range(kb):
                k = k0 + kk
                gb = int(gbtot[k])
                mask = mpool.tile([P, GB_MAX * BLK], TBL_DT, tag="mask")
                # one-hot mask via is_equal; rel is stored as duplicated
                # pairs so every operand's innermost dim is packed (DVE 2x).
                nc.vector.tensor_tensor(
                    out=mask[:, :gb * BLK].rearrange(
                        "p (g x y) -> p g x y", g=gb, y=2),
                    in0=iota_m[:, :gb * BLK].rearrange(
                        "p (g x y) -> p g x y", g=gb, y=2),
                    in1=rel_sb[:, relcol * 2:(relcol + gb) * 2].rearrange(
                        "p (g y) -> p g y", y=2).unsqueeze(2).to_broadcast(
                        [P, gb, BLK // 2, 2]),
                    op=mybir.AluOpType.is_equal,
                )
                pa = pa_pool.tile([C, BLK], f32, tag="pa")
                j = 0
                for w in range(NW):
                    for jj in range(int(g_uni[k, w])):
                        col = int(woff[w] + boff[w] + jj)
                        nc.tensor.matmul(
                            out=pa[:],
                            lhsT=g_tile[:, col * C:(col + 1) * C],
                            rhs=mask[:, j * BLK:(j + 1) * BLK],
                            start=(j == 0),
                            stop=(j == gb - 1),
                        )
                        j += 1
                agg = apool.tile([C, BLK], TBL_DT, tag="agg")
                nc.scalar.activation(agg[:], pa[:], mybir.ActivationFunctionType.Copy)
                pb = pb_pool.tile([BLK, C], f32, tag="pb")
                nc.tensor.matmul(out=pb[:], lhsT=agg[:], rhs=w16[:], start=True, stop=True)
                nc.vector.scalar_tensor_tensor(
                    out=ostage[:, kk * C:(kk + 1) * C],
                    in0=pb[:],
                    scalar=ivec_sb[:, k:k + 1],
                    in1=bias_tile[:],
                    op0=mybir.AluOpType.mult,
                    op1=mybir.AluOpType.add,
                )
                relcol += gb
                boff += g_uni[k]
            dram_view = out[k0 * BLK:k1 * BLK, :].rearrange(
                "(bb p) c -> p bb c", p=P
            )
            nc.sync.dma_start(
                out=dram_view,
                in_=ostage[:, :kb * C].rearrange("p (bb c) -> p bb c", bb=kb),
            )

    nc.compile()
    return nc


def _balance_caps(dw, caps):
    """Greedy maximin placement by descending degree + pairwise swap repair.
    Returns (members [NBLK, BLK], ok)."""
    order = np.argsort(-(dw.max(1) * 64 + dw.sum(1)), kind="stable")
    slack = caps.copy()
    slots = np.full(NBLK, BLK, np.int64)
    assign = np.empty(N_PAD, np.int64)
    BIG = -10**6
    for d in order:
        v = dw[d]
        score = (slack - v).min(axis=1)
        score[slots == 0] = BIG
        b = int(np.argmax(score))
        assign[d] = b
        slack[b] -= v
        slots[b] -= 1
    loads = caps - slack
    members = np.full((NBLK, BLK), -1, np.int64)
    cnt = np.zeros(NBLK, np.int64)
    for d in range(N_PAD):
        b = assign[d]
        members[b, cnt[b]] = d
        cnt[b] += 1
    rng = np.random.default_rng(0)
    it = 0
    while np.maximum(loads - caps, 0).sum() > 0 and it < 20000:
        it += 1
        ov = loads - caps
        b1, w1 = np.unravel_index(np.argmax(ov + rng.random(ov.shape) * 0.01), ov.shape)
        fixed = False
        for di in np.argsort(-dw[members[b1], w1])[:4]:
            d1 = members[b1, di]
            delta_all = dw[d1][None, None, :] - dw[members]
            nb2 = np.maximum(loads[:, None, :] + delta_all - caps[:, None, :], 0).sum(2)
            nb1 = np.maximum(loads[b1][None, None, :] - delta_all - caps[b1][None, None, :], 0).sum(2)
            cur = np.maximum(loads - caps, 0).sum(1)[:, None] \
                + np.maximum(loads[b1] - caps[b1], 0).sum()
            gain = cur - (nb1 + nb2)
            gain[b1, :] = -1
            b2, dj = np.unravel_index(np.argmax(gain), gain.shape)
            if gain[b2, dj] > 0:
                d2 = members[b2, dj]
                members[b1, di] = d2
                members[b2, dj] = d1
                loads[b1] += dw[d2] - dw[d1]
                loads[b2] += dw[d1] - dw[d2]
                fixed = True
                break
        if not fixed:
            break
    return members, bool((loads <= caps).all())


def _balance(dw):
    """Assign every dst id to a (block, lane) so that each (block, window)
    section's edge count fits a uniform per-block capacity.  Starts with one
    5-group overflow block per core and widens capacity if repair fails.
    Returns (members, g_uni)."""
    for extra in range(4):
        caps = np.full((NBLK, NW), 4 * BLK, np.int64)
        g_uni = np.full((NB, NW), 4, np.int64)
        for j in range(1 + extra):
            k = (OVF_K + j * 7) % NB
            caps[k::NB] = 5 * BLK
            g_uni[k] = 5
        members, ok = _balance_caps(dw, caps)
        if ok:
            return members, g_uni
    raise AssertionError("balance failed; residual overflow")


def _preprocess(feat, src, dst, W_mu, W_rho, bias_mu, bias_rho, W_eps, bias_eps):
    """Index-domain preprocessing + table pre-scaling. Returns (in_maps, key)."""
    src = np.asarray(src).astype(np.int64)
    dst = np.asarray(dst).astype(np.int64)
    feat = np.asarray(feat, dtype=np.float32)

    out_deg = np.bincount(src, minlength=N_NODES).astype(np.float32)
    o = 1.0 / np.sqrt(np.maximum(out_deg, 1.0))
    in_deg = np.bincount(dst, minlength=N_NODES)
    ivec_full = (1.0 / np.sqrt(np.maximum(in_deg, 1.0))).astype(np.float32)

    table = np.zeros((N_PAD, C), TBL_NP)
    table[:N_NODES] = (feat * o[:, None]).astype(TBL_NP)

    win = src // WROWS

    # --- balanced dst -> (block, lane) assignment ---------------------------
    dw = np.zeros((N_PAD, NW), np.int64)
    np.add.at(dw, (dst, win), 1)
    members, g_uni = _balance(dw)          # [NBLK, BLK] -> dst id
    blk_of = np.empty(N_PAD, np.int64)     # dst -> block
    lane_of = np.empty(N_PAD, np.int64)    # dst -> lane
    ar = np.arange(BLK)
    for b in range(NBLK):
        blk_of[members[b]] = b
        lane_of[members[b]] = ar

    # --- uniform structural layout ------------------------------------------
    gbtot = g_uni.sum(axis=1)
    G_TOTAL = int(gbtot.sum())
    GB_MAX = int(gbtot.max())
    sbs = _plan_superblocks(g_uni)
    key = (tuple(g_uni.ravel().tolist()), tuple(sbs))

    col_base = np.zeros((NB, NW), np.int64)
    colbase = 0
    for (k0, k1) in sbs:
        gw = g_uni[k0:k1].sum(axis=0)
        woff = np.concatenate([[0], np.cumsum(gw)])[:NW]
        run = np.zeros(NW, np.int64)
        for k in range(k0, k1):
            col_base[k] = colbase + woff + run
            run += g_uni[k]
        colbase += int(gw.sum())
    assert colbase == G_TOTAL
    rel_base = np.concatenate([[0], np.cumsum(gbtot)])[:NB]
    wg_off = np.concatenate([np.zeros((NB, 1), np.int64),
                             np.cumsum(g_uni, axis=1)[:, :NW - 1]], axis=1)

    # --- per-edge slot computation ------------------------------------------
    gb_edge = blk_of[dst]
    core = gb_edge // NB
    k_edge = gb_edge % NB
    seckey = (core * NB + k_edge) * NW + win
    order = np.argsort(seckey, kind="stable")
    sk = seckey[order]
    ss = src[order]
    lane_s = lane_of[dst[order]]
    cnt = np.bincount(seckey, minlength=N_CORES * NB * NW)
    starts = np.zeros(N_CORES * NB * NW + 1, np.int64)
    np.cumsum(cnt, out=starts[1:])
    pos = np.arange(len(ss), dtype=np.int64) - starts[sk]

    kr = (sk // NW) % NB
    wr = sk % NW
    cr = sk // (NW * NB)
    slots_core = G_TOTAL * BLK
    slot = cr * slots_core + col_base[kr, wr] * BLK + pos
    idx_all = np.zeros(N_CORES * slots_core, np.int16)     # pad -> row 0
    idx_all[slot] = (ss - wr * WROWS).astype(np.int16)

    rel_slot = cr * slots_core + (rel_base[kr] + wg_off[kr, wr]) * BLK + pos
    rel_all = np.full(N_CORES * slots_core, -1.0, TBL_NP)
    rel_all[rel_slot] = lane_s.astype(TBL_NP)

    ivec_pad = np.ones(N_PAD, np.float32)
    ivec_pad[:N_NODES] = ivec_full

    iota_np = np.tile(np.arange(BLK, dtype=TBL_NP), GB_MAX)[None, :].repeat(P, 0)
    iota_np = np.ascontiguousarray(iota_np)

    cc = np.ascontiguousarray
    in_maps = []
    for c in range(N_CORES):
        idx_c = idx_all[c * slots_core:(c + 1) * slots_core]
        idx_tile = np.tile(idx_c.reshape(-1, 16).T, (8, 1))
        rel_c = rel_all[c * slots_core:(c + 1) * slots_core].reshape(G_TOTAL, P).T
        rel2 = np.repeat(rel_c[:, :, None], 2, axis=2).reshape(P, G_TOTAL * 2)
        iv = ivec_pad[members[c * NB:(c + 1) * NB]].T   # [P, NB]
        in_maps.append({
            "table": table,
            "iota": iota_np,
            "idx": cc(idx_tile),
            "rel": cc(rel2),
            "ivec": cc(iv.astype(np.float32)),
            "w_mu": np.asarray(W_mu, np.float32),
            "w_rho": np.asarray(W_rho, np.float32),
            "w_eps": np.asarray(W_eps, np.float32),
            "b_mu": np.asarray(bias_mu, np.float32).reshape(1, C),
            "b_rho": np.asarray(bias_rho, np.float32).reshape(1, C),
            "b_eps": np.asarray(bias_eps, np.float32).reshape(1, C),
        })
    in_maps[0]["_members"] = members
    return in_maps, key


def kernel(**inputs) -> np.ndarray:
    in_maps, key = _preprocess(**inputs)
    members = in_maps[0].pop("_members")
    if key not in _CACHE:
        _CACHE[key] = _build_program(key)
    nc = _CACHE[key]
    res = run_bass_kernel_spmd(nc, in_maps, core_ids=list(range(N_CORES)))
    out_full = np.empty((N_PAD, C), np.float32)
    for c in range(N_CORES):
        rows = members[c * NB:(c + 1) * NB].ravel()
        out_full[rows] = res.results[c]["out"]
    return out_full[:N_NODES]
